# revision 1
# baseline (speedup 1.0000x reference)
"""GCN diag-encoder (2-layer SpMM) on 8 Trainium2 NeuronCores.

Strategy: the sparse adjacency (640K edges over 10K nodes, ~0.64% dense) is
materialized as a dense A^T matrix on the host; each per-layer
  out[dst] = sum_e vals[e] * x[src[e]]        (segment-sum SpMM)
becomes dense TensorEngine matmuls.  Each core owns a 1250-wide dst slice of
A^T (padded to 1280, uint8-quantized per dst column) and streams A^T k-tiles
from HBM with an inline u8->f16 cast in the DMA, in variable-size k-tile
groups (small first/last groups shorten the pipeline ramp and tail).

Layer 1 runs A-stationary — matmul(out=psum[dst,feat], lhsT=AT_tile[src,dst],
rhs=x_tile[src,feat]) — so the layer-1 output is already node-major: the
eviction is a fused tanh+dequant-scale pass on the scalar engine (scale is
per dst node = per partition) straight into the AllGather bounce.  PSUM
accumulation groups are per 2KiB bank while layer 1 writes four 512B ranges
per bank, so each bank is seeded by one full-width start=True zero matmul.
Layer 2 (PE-bound) runs X-stationary — matmul(out=psum[feat,dst],
lhsT=x1_tile[src,feat], rhs=AT_tile[src,dst]); its dequant scale (per dst =
per free element) and the final transpose are applied on the host.

Src nodes use a padded rank-block ordering (rank r owns slots
r*1280..r*1280+1279) so layer 2's AllGathered activations line up with the
SAME A arrangement layer 1 uses — the first RESG k-tile groups of A stay
resident in SBUF for layer 2, and layer 2 interleaves resident/streamed
groups so PE starts on the earliest-arriving x1 chunks while the remaining
A-stream DMAs land.  W0 is folded into x on the host; W1 is skipped on
device when it is all-ones (torch init), else applied via a broadcast
multiply.
"""

import numpy as np
import ml_dtypes

N = 10000          # nodes
D = 128            # feature dim
NCORES = 8
S = 1250           # dst nodes per core
SP = 1280          # padded dst per core (10 tiles of 128)
KT = 80            # contraction k-tiles (padded src rows = 10240)
NPAD = KT * 128    # 10240
GSIZES = (4,) * 20   # k-tiles per group
RESG = 12          # leading groups kept resident in SBUF for layer 2
BF16 = ml_dtypes.bfloat16

_PROG_CACHE = {}


def _groups():
    out = []
    k0 = 0
    for sz in GSIZES:
        out.append((k0, k0 + sz))
        k0 += sz
    assert k0 == KT
    return out


def _build_program(nocc=False, skip=(), u8=True, resg=RESG, abufs=4,
                   w1_ones=True, l2order="streamfirst", gsizes=GSIZES):
    import concourse.bacc as bacc
    import concourse.mybir as mybir
    from concourse import tile

    f32 = mybir.dt.float32
    f16 = mybir.dt.float16
    adt = mybir.dt.uint8 if u8 else f16
    grps = []
    _k0 = 0
    for _sz in gsizes:
        grps.append((_k0, _k0 + _sz))
        _k0 += _sz
    assert _k0 == KT
    maxg = max(k1 - k0 for k0, k1 in grps)

    nc = bacc.Bacc(
        "TRN2",
        target_bir_lowering=False,
        debug=False,
        enable_asserts=False,
        num_devices=1 if nocc else NCORES,
    )

    a = nc.dram_tensor("a", [KT, 128, SP], adt, kind="ExternalInput").ap()
    # f16 copy of the streamed (non-resident) k-range: layer 2 refetches it
    # on the sync HWDGE ring, FIFO-ordered behind the critical x1 loads
    ks0 = grps[resg][0] if resg < len(grps) else KT
    af = nc.dram_tensor(
        "af", [max(KT - ks0, 1), 128, SP], f16, kind="ExternalInput"
    ).ap()
    x0 = nc.dram_tensor("x0", [128, NPAD], f16, kind="ExternalInput").ap()
    # per-dst-node dequant scales, [slot p, tile t] layout
    csc = nc.dram_tensor("csc", [128, 10], f32, kind="ExternalInput").ap()
    # broadcast W1 row (only read when not w1_ones)
    w1b = nc.dram_tensor("w1b", [128, 128], f16, kind="ExternalInput").ap()
    out = nc.dram_tensor("out", [128, SP], f32, kind="ExternalOutput").ap()

    with tile.TileContext(nc) as tc:
        with (
            tc.tile_pool(name="xp", bufs=1) as xp,
            tc.tile_pool(name="ab", bufs=abufs) as apool,
            tc.tile_pool(name="res", bufs=1) as rpool,
            tc.tile_pool(name="ev", bufs=1) as ev,
            tc.tile_pool(name="ps", bufs=1, space="PSUM") as ps,
            tc.tile_pool(name="dr", bufs=1, space="DRAM") as dr,
        ):
            # x0 is dead once layer 1 finishes; share one slot for both
            x0s = xp.tile([128, NPAD], f16, tag="xs")
            x1s = xp.tile([128, NPAD], f16, tag="xs")
            cscs = xp.tile([128, 10], f32, tag="cscs")
            w1s = xp.tile([128, 128], f16, tag="w1s")
            zl = xp.tile([128, 512], f16, tag="zl")
            warm = xp.tile([128, 1], f32, tag="warm")
            nc.scalar.dma_start(cscs[:], csc)
            if not w1_ones:
                nc.scalar.dma_start(w1s[:], w1b)
            nc.vector.memset(zl[:], 0.0)
            # pre-load the ACT tanh table so the layer-1 eviction doesn't
            # pay the table load on the critical path
            nc.scalar.activation(
                warm[:], zl[:, 0:1], mybir.ActivationFunctionType.Tanh
            )

            agin = dr.tile([128, SP], f16)
            agout = dr.tile([NCORES * 128, SP], f16, addr_space="Shared")

            res_tiles = {}

            def fetch_group(gi, halves=1, via_f16=False):
                """DMA group gi of A into an SBUF tile (f16, cast if u8)."""
                k0, k1 = grps[gi]
                nk = k1 - k0
                if gi < resg:
                    ab = rpool.tile([128, nk * SP], f16, tag=f"res{gi}")
                    res_tiles[gi] = ab
                else:
                    ab = apool.tile([128, maxg * SP], f16, tag="ab")
                if "adma" in skip:
                    nc.gpsimd.dma_start(ab[:, 0:8], a[k0][:, 0:8])
                    return ab
                bounds = [k0 + (nk * h) // halves for h in range(halves + 1)]
                for b0, b1 in zip(bounds, bounds[1:]):
                    if b0 == b1:
                        continue
                    dst = ab[:, (b0 - k0) * SP:(b1 - k0) * SP].rearrange(
                        "p (k j) -> p k j", k=b1 - b0
                    )
                    if via_f16:
                        src = af[b0 - ks0:b1 - ks0].rearrange("k p j -> p k j")
                        nc.sync.dma_start(dst, src)
                    elif u8:
                        src = a[b0:b1].rearrange("k p j -> p k j")
                        nc.gpsimd.dma_start(dst, src)
                    else:
                        src = a[b0:b1].rearrange("k p j -> p k j")
                        nc.sync.dma_start(dst, src)
                return ab

            # ---- layer 1 (A-stationary; psum is [dst slot, feat]) ----
            psum1 = ps.tile([128, SP], f32, tag="acc1")
            for c0, cn in ((0, 512), (512, 512), (1024, 256)):
                nc.tensor.matmul(
                    psum1[:, c0:c0 + cn], zl[:, 0:128], zl[:, 0:cn],
                    start=True, stop=False,
                )
            for gi, (k0, k1) in enumerate(grps):
                nc.scalar.dma_start(
                    x0s[:, k0 * 128:k1 * 128], x0[:, k0 * 128:k1 * 128]
                )
                ab = fetch_group(gi, halves=2 if gi == 0 else 1)
                if gi < len(grps) - 1:
                    for k in range(k0, k1):
                        kk = k - k0
                        rhs = x0s[:, k * 128:(k + 1) * 128]
                        for t in range(10):
                            nc.tensor.matmul(
                                psum1[:, t * 128:(t + 1) * 128],
                                ab[:, kk * SP + t * 128:
                                   kk * SP + (t + 1) * 128],
                                rhs,
                                start=False, stop=False,
                            )
                else:
                    # final group t-outer: each dst range finishes early so
                    # the tanh eviction overlaps the remaining matmuls
                    for t in range(10):
                        for k in range(k0, k1):
                            kk = k - k0
                            nc.tensor.matmul(
                                psum1[:, t * 128:(t + 1) * 128],
                                ab[:, kk * SP + t * 128:
                                   kk * SP + (t + 1) * 128],
                                x0s[:, k * 128:(k + 1) * 128],
                                start=False,
                                stop=(k == KT - 1 and t in (3, 7, 9)),
                            )

            # evict layer 1: x1 = tanh(cs_dst * psum1) [* W1] on ACT, chunked
            # agin DMA so the AllGather input lands as soon as possible.
            agin_sb = ev.tile([128, SP], f16, tag="agin")
            for t in range(10):
                c0, c1 = t * 128, (t + 1) * 128
                nc.scalar.activation(
                    agin_sb[:, c0:c1], psum1[:, c0:c1],
                    mybir.ActivationFunctionType.Tanh,
                    scale=cscs[:, t:t + 1],
                )
                if not w1_ones:
                    nc.vector.tensor_mul(
                        agin_sb[:, c0:c1], agin_sb[:, c0:c1], w1s[:]
                    )
                nc.scalar.dma_start(agin[:, c0:c1], agin_sb[:, c0:c1])

            residents_pre = list(range(resg))
            streams_pre = list(range(resg, len(grps)))
            if l2order == "streamfirst":
                _order_preview = streams_pre[:abufs] + residents_pre + streams_pre[abufs:]
            elif l2order == "weave":
                _order_preview = []
                for i in range(2):
                    if i < len(streams_pre):
                        _order_preview.append(streams_pre[i])
                    if i < len(residents_pre):
                        _order_preview.append(residents_pre[i])
                _order_preview += residents_pre[2:] + streams_pre[2:]
            else:
                _order_preview = None

            if nocc:
                nc.scalar.dma_start(agout[0:128, :], agin[:])
            else:
                nc.gpsimd.collective_compute(
                    "AllGather",
                    mybir.AluOpType.bypass,
                    replica_groups=[list(range(NCORES))],
                    ins=[agin.opt()],
                    outs=[agout.opt()],
                )
            # agout rank blocks laid side by side in the free dim are exactly
            # layer-2's lhsT tiles in the same padded rank-block order A uses.
            rank_order = []
            for gi in _order_preview:
                k0, k1 = grps[gi]
                for r in ((k0 * 128) // SP, ((k1 * 128) - 1) // SP):
                    if r not in rank_order:
                        rank_order.append(r)
            for r in range(NCORES):
                if r not in rank_order:
                    rank_order.append(r)
            for r in rank_order:
                nc.sync.dma_start(
                    x1s[:, r * SP:(r + 1) * SP],
                    agout[r * 128:(r + 1) * 128, :],
                )

            # ---- layer 2 (X-stationary; psum is [feat, dst]) ----
            # Interleave: residents first (rank-0 x1 chunk arrives first),
            # streamed groups spread out so their DMAs pipeline through the
            # abufs slots while PE chews residents.
            psum2 = ps.tile([128, SP], f32, tag="acc2")
            residents = residents_pre
            streams = streams_pre
            if l2order == "streamfirst":
                order = streams[:abufs] + residents + streams[abufs:]
            elif l2order == "weave":
                # s0 r0 s1 r1 r2 ... then remaining streams at the tail
                order = []
                for i in range(2):
                    if i < len(streams):
                        order.append(streams[i])
                    if i < len(residents):
                        order.append(residents[i])
                order += residents[2:] + streams[2:]
            else:
                order = []
                ri, si = 0, 0
                pattern = [0, 0, 1, 0, 1, 0, 1, 0, 1, 0, 1, 0]  # 1 = stream
                for p in pattern[:len(grps)]:
                    if p and si < len(streams):
                        order.append(streams[si]); si += 1
                    elif ri < len(residents):
                        order.append(residents[ri]); ri += 1
                order += residents[ri:] + streams[si:]
                if order[-1] in streams:
                    for i in range(len(order) - 2, -1, -1):
                        if order[i] in residents:
                            order.append(order.pop(i))
                            break

            ob = ev.tile([128, SP], f32, tag="ob")
            first = True
            for oi, gi in enumerate(order):
                k0, k1 = grps[gi]
                ab = (res_tiles[gi] if gi < resg
                      else fetch_group(gi, via_f16=True))
                last_grp = oi == len(order) - 1
                if not last_grp:
                    for k in range(k0, k1):
                        kk = k - k0
                        lhsT = x1s[:, k * 128:(k + 1) * 128]
                        for c0, cn in ((0, 512), (512, 512), (1024, 256)):
                            nc.tensor.matmul(
                                psum2[:, c0:c0 + cn],
                                lhsT,
                                ab[:, kk * SP + c0: kk * SP + c0 + cn],
                                start=first, stop=False,
                            )
                        first = False
                else:
                    # final group: bank-outer so each psum2 bank completes
                    # (stop=True) early and its eviction overlaps the rest
                    for c0, cn in ((0, 512), (512, 512), (1024, 256)):
                        for k in range(k0, k1):
                            kk = k - k0
                            nc.tensor.matmul(
                                psum2[:, c0:c0 + cn],
                                x1s[:, k * 128:(k + 1) * 128],
                                ab[:, kk * SP + c0: kk * SP + c0 + cn],
                                start=False, stop=(k == k1 - 1),
                            )
                        nc.vector.tensor_copy(
                            ob[:, c0:c0 + cn], psum2[:, c0:c0 + cn]
                        )
                        nc.sync.dma_start(
                            out[:, c0:c0 + cn], ob[:, c0:c0 + cn]
                        )

    nc.compile()
    return nc


def get_program(nocc=False, skip=(), u8=True, resg=RESG, abufs=4,
                w1_ones=True, l2order="streamfirst", gsizes=GSIZES):
    key = ("nc", nocc, tuple(skip), u8, resg, abufs, w1_ones, l2order,
           tuple(gsizes))
    if key not in _PROG_CACHE:
        _PROG_CACHE[key] = _build_program(nocc, skip, u8, resg, abufs,
                                          w1_ones, l2order, gsizes)
    return _PROG_CACHE[key]


def _node_perm():
    """Padded rank-block src ordering: slot i <-> (rank r = i//1280,
    local q = i%1280); global node r*1250+q for q<1250, else pad."""
    i2 = np.arange(NPAD)
    r2 = i2 // SP
    loc = i2 % SP
    node = r2 * S + loc
    valid = loc < S
    return np.where(valid, node, 0), valid


def build_in_maps(x, src, dst, vals, W, u8=True):
    """Host-side prep: dense A^T shard (u8 per-column quantized) + x0."""
    import scipy.sparse as sp

    x = np.asarray(x, np.float32)
    src = np.asarray(src, np.int64)
    dst = np.asarray(dst, np.int64)
    vals = np.asarray(vals, np.float32)
    W = np.asarray(W, np.float32)

    # A[dst, src] = sum of vals  ->  we build AT[src, dst]
    AT = sp.coo_matrix((vals, (src, dst)), shape=(N, N)).toarray()

    node2, valid2 = _node_perm()

    xw = x * W[0][None, :]
    x0p = np.zeros((NPAD, D), np.float32)
    x0p[valid2] = xw[node2[valid2]]
    x0h = np.ascontiguousarray(
        x0p.reshape(KT, 128, D).transpose(1, 0, 2).reshape(128, KT * D)
    ).astype(np.float16)

    w1brow = np.ascontiguousarray(
        np.tile(W[1][None, :], (128, 1))
    ).astype(np.float16)

    in_maps = []
    steps = []
    for c in range(NCORES):
        ATc = AT[:, c * S:(c + 1) * S]  # [N, S] float32
        colmax = np.maximum(ATc.max(axis=0), 1e-9)
        step = colmax / 255.0
        if u8:
            Aq = np.clip(np.rint(ATc * (1.0 / step)[None, :]), 0, 255).astype(
                np.uint8
            )
        else:
            Aq = (ATc * (1.0 / step)[None, :]).astype(np.float16)
        Ap = np.zeros((NPAD, SP), Aq.dtype)
        Ap[valid2, :S] = Aq[node2[valid2]]
        step_pad = np.zeros(SP, np.float32)
        step_pad[:S] = step
        steps.append(step_pad)
        # csc[p, t] = dequant scale of dst slot t*128+p
        csc_tile = np.ascontiguousarray(step_pad.reshape(10, 128).T).astype(
            np.float32
        )
        a3 = np.ascontiguousarray(Ap.reshape(KT, 128, SP))
        ks0 = sum(GSIZES[:RESG])
        in_maps.append(
            {
                "a": a3,
                "af": np.ascontiguousarray(a3[ks0:].astype(np.float16)),
                "x0": x0h,
                "csc": csc_tile,
                "w1b": w1brow,
            }
        )
    return in_maps, steps


def assemble_output(results, steps):
    outs = []
    for c in range(NCORES):
        ot = np.asarray(results[c]["out"], np.float32)  # [128, SP] feat-major
        ot = ot * steps[c][None, :]  # per-dst dequant (layer-2)
        outs.append(ot[:, :S].T)
    return np.ascontiguousarray(np.concatenate(outs, axis=0))


def kernel(x, src, dst, vals, W):
    from concourse import bass_utils

    w1_ones = bool(np.all(np.asarray(W)[1] == 1.0))
    nc = get_program(w1_ones=w1_ones)
    in_maps, steps = build_in_maps(x, src, dst, vals, W)
    # The axon terminal can wedge when a different program was loaded
    # earlier in its lifetime; after the crash the terminal restarts and a
    # retry succeeds.  Back off progressively to ride out the restart.
    import time as _time

    last_err = None
    for sleep_s in (10.0, 30.0, 60.0, 0.0):
        try:
            res = bass_utils.run_bass_kernel_spmd(
                nc, in_maps, core_ids=list(range(NCORES))
            )
            return assemble_output(res.results, steps)
        except Exception as e:  # noqa: BLE001
            last_err = e
            _time.sleep(sleep_s)
    raise last_err



# revision 75
# speedup vs baseline: 1.3700x; 1.3700x over previous
"""GCN diag-encoder (2-layer SpMM) on 8 Trainium2 NeuronCores.

Strategy: the sparse adjacency (640K edges over 10K nodes, ~0.64% dense) is
materialized as a dense A^T on the host; each per-layer
  out[dst] = sum_e vals[e] * x[src[e]]        (segment-sum SpMM)
becomes dense TensorEngine matmuls.  Each core owns a 1250-wide dst slice of
A^T (padded to 1280 = 10 tiles of 128 dst slots).

A^T is kept FULLY RESIDENT in SBUF in one-byte-per-element form, streamed
from HBM exactly once (~13 MB, ~36 us of DMA), so neither layer re-streams
it.  Two per-column quantizations split the dst tiles:
  - tiles 0-3  (slots    0- 511): fp8 e3m4, consumed by the PE directly
    (both as layer-1 lhsT and layer-2 moving operand; fp8e3 runs at the
    same 1 row/cycle as f16 in the cost model),
  - tiles 4-9  (slots 512-1279): uint8 (255-level, ~4x lower quantization
    error than e3m4), cast u8->f16 into small staging rings by the
    otherwise-idle Activation/Vector/GpSimd engines right before use.
The mix keeps the end-to-end relative error ~1.4e-2 (vs 2.2e-2 for pure
e3m4) while the DMA pool only ever moves one byte per A element.  The
baseline instead streamed A u8 with an inline u8->f16 cast DMA — which the
cost model charges at the 2-byte WRITE side — plus an f16 re-stream for
layer 2, making it DMA-bound (121 us DMA vs 92 us PE).  Here DMA drops to
~55 us and the kernel is PE-bound (~85 us of matmul rows).

Layer 1 runs A-stationary — matmul(out=psum[dst,feat], lhsT=AT_tile[src,dst],
rhs=x_tile[src,feat]) — so the layer-1 output is already node-major: the
eviction is a fused tanh+dequant-scale pass on the scalar engine (scale is
per dst node = per partition) straight into the AllGather bounce.  PSUM
accumulation groups are per 2KiB bank while layer 1 writes four 512B ranges
per bank, so each bank is seeded by one full-width start=True zero matmul.
Layer 2 (PE-bound) runs X-stationary — matmul(out=psum[feat,dst],
lhsT=x1_tile[src,feat], rhs=AT_chunk[src,dst]); its chunks (0,512)=e3m4,
(512,512)+(1024,256)=casted-f16 are exactly PSUM-bank aligned.  The layer-2
dequant scale (per dst = per free element) and the final transpose are
applied on the host.

Src nodes use a padded rank-block ordering (rank r owns slots
r*1280..r*1280+1279) so layer 2's AllGathered activations line up with the
SAME A arrangement layer 1 uses.  The layer-2 u8->f16 staging casts have no
dependency on the AllGather, so they run ahead during the layer boundary
and the PE restarts on rank 0's x1 chunk as soon as it lands.  W0 is folded
into x on the host; W1 is skipped on device when it is all-ones (torch
init), else applied via a broadcast multiply.
"""

import numpy as np
import ml_dtypes

N = 10000          # nodes
D = 128            # feature dim
NCORES = 8
S = 1250           # dst nodes per core
SP = 1280          # padded dst per core (10 tiles of 128)
KT = 80            # contraction k-tiles (padded src rows = 10240)
NPAD = KT * 128    # 10240
GSIZE = 4          # k-tiles per DMA/cast group
NG = KT // GSIZE   # 20 groups
NE3 = 4            # leading dst tiles stored as fp8 e3m4 (slots 0-511)
WE3 = NE3 * 128            # 512
WU8 = SP - WE3             # 768
E3_TARGET = 7.8    # colmax maps to ~7.8 so values sit in e3m4's sweet spot
BF16 = ml_dtypes.bfloat16

_PROG_CACHE = {}


def _build_program(nocc=False, w1_ones=True, s1bufs=12, s2bufs=16,
                   l1_tail=4, filler=40, prefill=12):
    import concourse.bacc as bacc
    import concourse.mybir as mybir
    from concourse import tile

    f32 = mybir.dt.float32
    f16 = mybir.dt.float16
    e3 = mybir.dt.float8e3
    u8 = mybir.dt.uint8

    nc = bacc.Bacc(
        "TRN2",
        target_bir_lowering=False,
        debug=False,
        enable_asserts=False,
        num_devices=1 if nocc else NCORES,
    )

    au = nc.dram_tensor("au", [KT, 128, WU8], u8, kind="ExternalInput").ap()
    ae = nc.dram_tensor("ae", [KT, 128, WE3], e3, kind="ExternalInput").ap()
    x0 = nc.dram_tensor("x0", [128, NPAD], f16, kind="ExternalInput").ap()
    # per-dst-node dequant scales, [slot p, tile t] layout
    csc = nc.dram_tensor("csc", [128, 10], f32, kind="ExternalInput").ap()
    # broadcast W1 row (only read when not w1_ones)
    w1b = nc.dram_tensor("w1b", [128, 128], f16, kind="ExternalInput").ap()
    # f16 output (values ~±26, rel tolerance 2e-2 — f16 rounding is noise);
    # written straight from PSUM, halving the final eviction DMA traffic
    out = nc.dram_tensor("out", [128, SP], f16, kind="ExternalOutput").ap()

    with tile.TileContext(nc) as tc:
        with (
            tc.tile_pool(name="xp", bufs=1) as xp,
            tc.tile_pool(name="s1", bufs=s1bufs) as s1pool,
            tc.tile_pool(name="s2", bufs=s2bufs) as s2pool,
            tc.tile_pool(name="ev", bufs=1) as ev,
            tc.tile_pool(name="ps", bufs=1, space="PSUM") as ps,
            tc.tile_pool(name="dr", bufs=1, space="DRAM") as dr,
        ):
            # x0 is dead once layer 1 finishes; share one slot for both
            x0s = xp.tile([128, NPAD], f16, tag="xs")
            x1s = xp.tile([128, NPAD], f16, tag="xs")
            aur = xp.tile([128, KT * WU8], u8, tag="aur")
            aer = xp.tile([128, KT * WE3], e3, tag="aer")
            cscs = xp.tile([128, 10], f32, tag="cscs")
            cscw = xp.tile([128, SP], f32, tag="cscw")
            w1s = xp.tile([128, 128], f16, tag="w1s")
            zl = xp.tile([128, 512], f16, tag="zl")
            warm = xp.tile([128, 1], f32, tag="warm")
            nc.scalar.dma_start(cscs[:], csc)
            if not w1_ones:
                nc.scalar.dma_start(w1s[:], w1b)
            nc.vector.memset(zl[:], 0.0)
            # pre-load the ACT tanh table so the layer-1 eviction doesn't
            # pay the table load on the critical path
            nc.scalar.activation(
                warm[:], zl[:, 0:1], mybir.ActivationFunctionType.Tanh
            )
            # broadcast csc[p, t] -> cscw[p, t*128+f] on the idle gpsimd
            # engine (a 5KB/partition csc DMA would delay the A stream)
            for t in range(10):
                nc.gpsimd.tensor_scalar_add(
                    cscw[:, t * 128:(t + 1) * 128],
                    zl[:].bitcast(f32)[:, 0:128],
                    cscs[:, t:t + 1],
                )

            agin = dr.tile([128, SP], f16)
            agout = dr.tile([NCORES * 128, SP], f16, addr_space="Shared")

            def fetch_range(b0, b1):
                # ae first: ktile t-order hits the e3m4 tiles (t<4) first
                dste = aer[:, b0 * WE3:b1 * WE3].rearrange(
                    "p (k j) -> p k j", k=b1 - b0
                )
                nc.sync.dma_start(
                    dste, ae[b0:b1].rearrange("k p j -> p k j")
                )
                dstu = aur[:, b0 * WU8:b1 * WU8].rearrange(
                    "p (k j) -> p k j", k=b1 - b0
                )
                nc.sync.dma_start(
                    dstu, au[b0:b1].rearrange("k p j -> p k j")
                )

            def fetch_groups(gi, halves=1):
                """DMA group gi of au + ae on the sync ring."""
                k0 = gi * GSIZE
                bounds = [k0 + (GSIZE * h) // halves for h in range(halves + 1)]
                for b0, b1 in zip(bounds, bounds[1:]):
                    if b0 != b1:
                        fetch_range(b0, b1)

            # rotation weighted by engine copy speed (ACT 0.83, DVE 1.04,
            # GpSimd 1.39 ns/row): ACT 2/5, DVE 2/5, Pool 1/5
            cast_engines = (nc.scalar, nc.vector, nc.scalar, nc.vector,
                            nc.gpsimd)

            def cast_u8(k, pool, tag, eng=None):
                """u8->f16 cast of AUR ktile k on a rotating engine."""
                st = pool.tile([128, WU8], f16, tag=tag)
                if eng is None:
                    eng = cast_engines[k % 5]
                src = aur[:, k * WU8:(k + 1) * WU8]
                if eng is nc.scalar:
                    nc.scalar.activation(
                        st[:], src, mybir.ActivationFunctionType.Copy
                    )
                else:
                    eng.tensor_copy(st[:], src)
                return st

            # ---- layer 1 (A-stationary; psum is [dst slot, feat]) ----
            # one PSUM tile per 2KiB bank so Tile scopes the eviction's RAW
            # dependency to that bank's stop=True matmul (a single [128,SP]
            # tile would serialize every eviction behind the LAST bank).
            # all PSUM tiles are full 2KiB banks: start=True resets the WHOLE
            # bank, so half-bank tiles sharing a bank would wipe each other
            p1 = [
                ps.tile([128, 512], f32, tag="acc1a", name="p1a"),
                ps.tile([128, 512], f32, tag="acc1b", name="p1b"),
                ps.tile([128, 512], f32, tag="acc1c", name="p1c"),
            ]
            pf = ps.tile([128, 512], f32, tag="pfill", name="pfill")
            # seed each layer-1 bank with one full-width start=True zero
            # matmul: the real matmuls write four 512B ranges per bank with
            # start=False (a per-range start=True would reset the whole bank
            # and erase the sibling ranges' first contributions)
            for pt in p1:
                nc.tensor.matmul(
                    pt[:], zl[:, 0:128], zl[:, 0:512],
                    start=True, stop=False,
                )
            # pre-filler: throwaway matmuls the scheduler hoists to t~1us.
            # They warm the tensor engine's p-state ramp clock and absorb
            # the first stream's ~4.5us DMA latency, so the real layer-1
            # matmuls start at full speed with a stream lead built up —
            # at DMA/PE parity a stall early in layer 1 is never recovered.
            for i in range(prefill):
                nc.tensor.matmul(
                    pf[:], zl[:, 0:128], zl[:, 0:512],
                    start=True, stop=(i == prefill - 1),
                )

            def l1_psum(t):
                b = min(t // 4, 2)
                return p1[b][:, (t - b * 4) * 128:(t - b * 4 + 1) * 128]

            def l1_lhsT(k, t, st):
                if t < NE3:
                    return aer[:, k * WE3 + t * 128:k * WE3 + (t + 1) * 128]
                o = (t - NE3) * 128
                return st[:, o:o + 128]

            kt0 = KT - l1_tail  # start of the t-outer eviction tail
            for gi in range(NG):
                k0, k1 = gi * GSIZE, (gi + 1) * GSIZE
                if k0 == 0:
                    # startup order: first A half-group, tiny x0 chunk, rest
                    # — the first matmul's operands land as early as possible
                    fetch_range(0, 2)
                    nc.sync.dma_start(
                        x0s[:, 0:2 * 128], x0[:, 0:2 * 128]
                    )
                    fetch_range(2, 4)
                    nc.sync.dma_start(
                        x0s[:, 2 * 128:8 * 128], x0[:, 2 * 128:8 * 128]
                    )
                else:
                    if k0 % 8 == 0:
                        # x0 streamed in 8-ktile chunks (fewer DMAs -> less
                        # HWDGE/sem overhead on the shared rings)
                        nc.sync.dma_start(
                            x0s[:, k0 * 128:(k0 + 8) * 128],
                            x0[:, k0 * 128:(k0 + 8) * 128],
                        )
                    fetch_groups(gi)
                if k0 >= kt0:
                    continue
                for k in range(k0, k1):
                    st = cast_u8(k, s1pool, "s1")
                    rhs = x0s[:, k * 128:(k + 1) * 128]
                    for t in range(10):
                        nc.tensor.matmul(
                            l1_psum(t),
                            l1_lhsT(k, t, st),
                            rhs,
                            start=False, stop=False,
                        )

            # evict layer 1: x1 = tanh(cs_dst * psum) [* W1] on ACT (scale is
            # per partition, fused into the activation) into the AllGather
            # bounce, one agin DMA per PSUM bank.
            agin_sb = ev.tile([128, SP], f16, tag="agin")
            ob = ev.tile([128, SP], f32, tag="ob")
            obh = ev.tile([128, SP], f16, tag="obh")

            def evict_l1_bank(b0, b1):
                c0, c1 = b0 * 128, b1 * 128
                b = b0 // 4
                nc.vector.tensor_mul(
                    ob[:, c0:c1], p1[b][:, 0:c1 - c0], cscw[:, c0:c1]
                )
                nc.scalar.activation(
                    agin_sb[:, c0:c1], ob[:, c0:c1],
                    mybir.ActivationFunctionType.Tanh,
                )
                if not w1_ones:
                    for t in range(b0, b1):
                        nc.vector.tensor_mul(
                            agin_sb[:, t * 128:(t + 1) * 128],
                            agin_sb[:, t * 128:(t + 1) * 128], w1s[:],
                        )
                nc.sync.dma_start(agin[:, c0:c1], agin_sb[:, c0:c1])

            # final l1_tail ktiles bank-outer: each dst bank finishes
            # (stop=True) early and its tanh+agin DMA overlap the remaining
            # banks' matmuls; only the last (quarter-size) bank trails.
            # These casts gate the whole tail (its first t-row reads every
            # tail ktile), so pin them to the two fastest engines.
            tail_engs = (nc.scalar, nc.vector)
            sts = {
                k: cast_u8(k, s1pool, "s1", eng=tail_engs[k % 2])
                for k in range(kt0, KT)
            }
            for b0, b1 in ((0, 4), (4, 8), (8, 10)):
                for t in range(b0, b1):
                    for k in range(kt0, KT):
                        nc.tensor.matmul(
                            l1_psum(t),
                            l1_lhsT(k, t, sts[k]),
                            x0s[:, k * 128:(k + 1) * 128],
                            start=False,
                            stop=(k == KT - 1),
                        )
                evict_l1_bank(b0, b1)

            # layer-2 staging casts have no dependency on the collective:
            # pre-issue a ring's worth so the casters fill them during the
            # layer boundary while the AllGather is in flight.
            s2_pre = {k: cast_u8(k, s2pool, "s2") for k in range(s2bufs)}

            # Keep the PE busy across the layer boundary with throwaway
            # matmuls into a scratch PSUM bank: an idle gap here resets the
            # tensor engine's p-state ramp and the first ~3 us of layer 2
            # would run at half/quarter speed (costs ~9 us).  The filler
            # runs while the AllGather + x1 readback are in flight and is
            # sized to end just before rank 0's x1 lands.
            if filler:
                # anchor on the last eviction bank's tanh output so the
                # scheduler cannot hoist the filler before the boundary
                for i in range(filler):
                    nc.tensor.matmul(
                        pf[:], agin_sb[:, 0:128], zl[:, 0:512],
                        start=True, stop=(i == filler - 1),
                    )

            if nocc:
                nc.sync.dma_start(agout[0:128, :], agin[:])
            else:
                nc.gpsimd.collective_compute(
                    "AllGather",
                    mybir.AluOpType.bypass,
                    replica_groups=[list(range(NCORES))],
                    ins=[agin.opt()],
                    outs=[agout.opt()],
                )
            # agout rank blocks laid side by side in the free dim are exactly
            # layer-2's lhsT tiles in the same padded rank-block order A uses.
            # In the nocc twin only rows 0:128 of agout are written; read all
            # ranks from there so every readback carries the RAW dependency
            # on the collective stand-in (otherwise the sim fires them early
            # and they congest the HWDGE ring right at the layer boundary,
            # which the real program's post-collective readbacks never do).
            # Rank 0 is split so layer 2 restarts on its first ktiles as soon
            # as a quarter-shard lands.
            def ag_src(r):
                return 0 if nocc else r * 128

            nc.sync.dma_start(
                x1s[:, 0:256], agout[ag_src(0):ag_src(0) + 128, 0:256]
            )
            nc.sync.dma_start(
                x1s[:, 256:SP], agout[ag_src(0):ag_src(0) + 128, 256:SP]
            )
            for r in range(1, NCORES):
                nc.sync.dma_start(
                    x1s[:, r * SP:(r + 1) * SP],
                    agout[ag_src(r):ag_src(r) + 128, :],
                )

            # ---- layer 2 (X-stationary; psum is [feat, dst]) ----
            # again one PSUM tile per bank; chunk boundaries are bank-aligned
            # ((0,512) e3m4 direct, (512,512)+(1024,256) casted-f16)
            p2 = [
                ps.tile([128, 512], f32, tag="acc2a", name="p2a"),
                ps.tile([128, 512], f32, tag="acc2b", name="p2b"),
                ps.tile([128, 512], f32, tag="acc2c", name="p2c"),
            ]

            def l2_chunks(k, st):
                yield 0, p2[0][:, 0:512], aer[:, k * WE3:(k + 1) * WE3]
                yield 512, p2[1][:, 0:512], st[:, 0:512]
                yield 1024, p2[2][:, 0:256], st[:, 512:768]

            def l2_cast(k):
                if k in s2_pre:
                    return s2_pre[k]
                return cast_u8(k, s2pool, "s2")

            for k in range(KT - GSIZE):
                st = l2_cast(k)
                lhsT = x1s[:, k * 128:(k + 1) * 128]
                for c0, pt, rhs in l2_chunks(k, st):
                    nc.tensor.matmul(
                        pt[:], lhsT, rhs,
                        start=(k == 0), stop=False,
                    )
            # final group: bank-outer so each psum2 bank completes
            # (stop=True) early and its eviction overlaps the rest
            kf = KT - GSIZE
            sts = {kk: l2_cast(kk) for kk in range(kf, KT)}
            chunks = {kk: list(l2_chunks(kk, sts[kk])) for kk in range(kf, KT)}
            for ci in range(3):
                for kk in range(kf, KT):
                    c0, pt, rhs = chunks[kk][ci]
                    nc.tensor.matmul(
                        pt[:], x1s[:, kk * 128:(kk + 1) * 128], rhs,
                        start=False, stop=(kk == KT - 1),
                    )
                c0, pt, _ = chunks[kf][ci]
                cn = 512 if ci < 2 else 256
                nc.vector.tensor_copy(obh[:, c0:c0 + cn], pt[:])
                nc.sync.dma_start(out[:, c0:c0 + cn], obh[:, c0:c0 + cn])

    nc.compile()
    return nc


def get_program(nocc=False, w1_ones=True, **kw):
    key = (nocc, w1_ones, tuple(sorted(kw.items())))
    if key not in _PROG_CACHE:
        _PROG_CACHE[key] = _build_program(nocc, w1_ones, **kw)
    return _PROG_CACHE[key]


def _node_perm():
    """Padded rank-block src ordering: slot i <-> (rank r = i//1280,
    local q = i%1280); global node r*1250+q for q<1250, else pad."""
    i2 = np.arange(NPAD)
    r2 = i2 // SP
    loc = i2 % SP
    node = r2 * S + loc
    valid = loc < S
    return np.where(valid, node, 0), valid


def build_in_maps(x, src, dst, vals, W):
    """Host-side prep: dense A^T shard (e3m4 + u8 per-column quantized)."""
    import scipy.sparse as sp

    x = np.asarray(x, np.float32)
    src = np.asarray(src, np.int64)
    dst = np.asarray(dst, np.int64)
    vals = np.asarray(vals, np.float32)
    W = np.asarray(W, np.float32)

    # A[dst, src] = sum of vals  ->  we build AT[src, dst]
    AT = sp.coo_matrix((vals, (src, dst)), shape=(N, N)).toarray()

    node2, valid2 = _node_perm()

    xw = x * W[0][None, :]
    x0p = np.zeros((NPAD, D), np.float32)
    x0p[valid2] = xw[node2[valid2]]
    x0h = np.ascontiguousarray(
        x0p.reshape(KT, 128, D).transpose(1, 0, 2).reshape(128, KT * D)
    ).astype(np.float16)

    w1brow = np.ascontiguousarray(
        np.tile(W[1][None, :], (128, 1))
    ).astype(np.float16)

    in_maps = []
    steps = []
    for c in range(NCORES):
        ATc = AT[:, c * S:(c + 1) * S]  # [N, S] float32
        colmax = np.maximum(ATc.max(axis=0), 1e-9)
        # permute + pad src rows once, in f32
        Ap = np.zeros((NPAD, SP), np.float32)
        Ap[valid2, :S] = ATc[node2[valid2]]
        # dequant scale per padded slot
        scale_pad = np.zeros(SP, np.float32)
        cm_pad = np.zeros(SP, np.float32)
        cm_pad[:S] = colmax
        cm_pad[S:] = 1.0
        # e3m4 tiles: slots [0, WE3)
        sc_e3 = E3_TARGET / np.maximum(cm_pad[:WE3], 1e-9)
        Ae = (Ap[:, :WE3] * sc_e3[None, :]).astype(ml_dtypes.float8_e3m4)
        scale_pad[:WE3] = 1.0 / sc_e3
        # u8 tiles: slots [WE3, SP)
        step = cm_pad[WE3:] / 255.0
        Au = np.clip(
            np.rint(Ap[:, WE3:] * (1.0 / step)[None, :]), 0, 255
        ).astype(np.uint8)
        scale_pad[WE3:] = step
        steps.append(scale_pad)
        # csc[p, t] = dequant scale of dst slot t*128+p
        csc_tile = np.ascontiguousarray(
            scale_pad.reshape(10, 128).T
        ).astype(np.float32)
        in_maps.append(
            {
                "au": np.ascontiguousarray(Au.reshape(KT, 128, WU8)),
                "ae": np.ascontiguousarray(Ae.reshape(KT, 128, WE3)),
                "x0": x0h,
                "csc": csc_tile,
                "w1b": w1brow,
            }
        )
    return in_maps, steps


def assemble_output(results, steps):
    outs = []
    for c in range(NCORES):
        ot = np.asarray(results[c]["out"], np.float32)  # [128, SP] feat-major
        ot = ot * steps[c][None, :]  # per-dst dequant (layer-2)
        outs.append(ot[:, :S].T)
    return np.ascontiguousarray(np.concatenate(outs, axis=0))


def kernel(x, src, dst, vals, W):
    from concourse import bass_utils

    w1_ones = bool(np.all(np.asarray(W)[1] == 1.0))
    nc = get_program(w1_ones=w1_ones)
    in_maps, steps = build_in_maps(x, src, dst, vals, W)
    # The axon terminal can wedge when a different program was loaded
    # earlier in its lifetime; after the crash the terminal restarts and a
    # retry succeeds.  Back off progressively to ride out the restart.
    import time as _time

    last_err = None
    for sleep_s in (10.0, 30.0, 60.0, 0.0):
        try:
            res = bass_utils.run_bass_kernel_spmd(
                nc, in_maps, core_ids=list(range(NCORES))
            )
            return assemble_output(res.results, steps)
        except Exception as e:  # noqa: BLE001
            last_err = e
            _time.sleep(sleep_s)
    raise last_err


# revision 80
# speedup vs baseline: 1.3899x; 1.0146x over previous
"""GCN diag-encoder (2-layer SpMM) on 8 Trainium2 NeuronCores.

Strategy: the sparse adjacency (640K edges over 10K nodes, ~0.64% dense) is
materialized as a dense A^T on the host; each per-layer
  out[dst] = sum_e vals[e] * x[src[e]]        (segment-sum SpMM)
becomes dense TensorEngine matmuls.  Each core owns a 1250-wide dst slice of
A^T (padded to 1280 = 10 tiles of 128 dst slots).

A^T is kept FULLY RESIDENT in SBUF in one-byte-per-element form, streamed
from HBM exactly once (~13 MB, ~36 us of DMA), so neither layer re-streams
it.  Two per-column quantizations split the dst tiles:
  - tiles 0-3  (slots    0- 511): fp8 e3m4, consumed by the PE directly
    (both as layer-1 lhsT and layer-2 moving operand; fp8e3 runs at the
    same 1 row/cycle as f16 in the cost model),
  - tiles 4-9  (slots 512-1249; the 30 pad slots are dropped): uint8
    (255-level, ~4x lower quantization error than e3m4), cast u8->f16 into
    small staging rings by the otherwise-idle Activation/Vector/GpSimd
    engines right before use.
The mix keeps the end-to-end relative error ~1.4e-2 (vs 2.2e-2 for pure
e3m4) while the DMA pool only ever moves one byte per A element.  The
baseline instead streamed A u8 with an inline u8->f16 cast DMA — which the
cost model charges at the 2-byte WRITE side — plus an f16 re-stream for
layer 2, making it DMA-bound (121 us DMA vs 92 us PE).  Here DMA drops to
~50 us and the kernel is PE-bound (~84 us of matmul rows).  Throwaway
"filler" matmuls keep the tensor engine busy at program start (absorbing
the stream's pipeline-fill latency) and across the AllGather boundary —
an idle PE gap resets the cost model's p-state ramp and the next ~3us of
matmuls would run at half speed.

Layer 1 runs A-stationary — matmul(out=psum[dst,feat], lhsT=AT_tile[src,dst],
rhs=x_tile[src,feat]) — so the layer-1 output is already node-major: the
eviction is a fused tanh+dequant-scale pass on the scalar engine (scale is
per dst node = per partition) straight into the AllGather bounce.  PSUM
accumulation groups are per 2KiB bank while layer 1 writes four 512B ranges
per bank, so each bank is seeded by one full-width start=True zero matmul.
Layer 2 (PE-bound) runs X-stationary — matmul(out=psum[feat,dst],
lhsT=x1_tile[src,feat], rhs=AT_chunk[src,dst]); its chunks (0,512)=e3m4,
(512,512)+(1024,226)=casted-f16 are PSUM-bank aligned.  The layer-2
dequant scale (per dst = per free element) and the final transpose are
applied on the host; the output travels as f16 (values ~±26 against a
2e-2 relative tolerance).

Src nodes use a padded rank-block ordering (rank r owns slots
r*1280..r*1280+1279) so layer 2's AllGathered activations line up with the
SAME A arrangement layer 1 uses.  The layer-2 u8->f16 staging casts have no
dependency on the AllGather, so they run ahead during the layer boundary
and the PE restarts on rank 0's x1 chunk as soon as it lands.  W0 is folded
into x on the host; W1 is skipped on device when it is all-ones (torch
init), else applied via a broadcast multiply.
"""

import numpy as np
import ml_dtypes

N = 10000          # nodes
D = 128            # feature dim
NCORES = 8
S = 1250           # dst nodes per core
SP = 1280          # padded dst per core (10 tiles of 128)
KT = 80            # contraction k-tiles (padded src rows = 10240)
NPAD = KT * 128    # 10240
GSIZE = 4          # k-tiles per DMA/cast group
NG = KT // GSIZE   # 20 groups
NE3 = 4            # leading dst tiles stored as fp8 e3m4 (slots 0-511)
WE3 = NE3 * 128            # 512
WU8 = S - WE3              # 738 (real dst only; pad slots 1250-1279 dropped)
E3_TARGET = 7.8    # colmax maps to ~7.8 so values sit in e3m4's sweet spot
BF16 = ml_dtypes.bfloat16

_PROG_CACHE = {}


def _build_program(nocc=False, w1_ones=True, s1bufs=12, s2bufs=16,
                   l1_tail=4, filler=40, prefill=12):
    import concourse.bacc as bacc
    import concourse.mybir as mybir
    from concourse import tile

    f32 = mybir.dt.float32
    f16 = mybir.dt.float16
    e3 = mybir.dt.float8e3
    u8 = mybir.dt.uint8

    nc = bacc.Bacc(
        "TRN2",
        target_bir_lowering=False,
        debug=False,
        enable_asserts=False,
        num_devices=1 if nocc else NCORES,
    )

    au = nc.dram_tensor("au", [KT, 128, WU8], u8, kind="ExternalInput").ap()
    ae = nc.dram_tensor("ae", [KT, 128, WE3], e3, kind="ExternalInput").ap()
    x0 = nc.dram_tensor("x0", [128, NPAD], f16, kind="ExternalInput").ap()
    # per-dst-node dequant scales, [slot p, tile t] layout
    csc = nc.dram_tensor("csc", [128, 10], f32, kind="ExternalInput").ap()
    # broadcast W1 row (only read when not w1_ones)
    w1b = nc.dram_tensor("w1b", [128, 128], f16, kind="ExternalInput").ap()
    # f16 output (values ~±26, rel tolerance 2e-2 — f16 rounding is noise);
    # written straight from PSUM, halving the final eviction DMA traffic
    out = nc.dram_tensor("out", [128, SP], f16, kind="ExternalOutput").ap()

    with tile.TileContext(nc) as tc:
        with (
            tc.tile_pool(name="xp", bufs=1) as xp,
            tc.tile_pool(name="s1", bufs=s1bufs) as s1pool,
            tc.tile_pool(name="s2", bufs=s2bufs) as s2pool,
            tc.tile_pool(name="ev", bufs=1) as ev,
            tc.tile_pool(name="ps", bufs=1, space="PSUM") as ps,
            tc.tile_pool(name="dr", bufs=1, space="DRAM") as dr,
        ):
            # x0 is dead once layer 1 finishes; share one slot for both
            x0s = xp.tile([128, NPAD], f16, tag="xs")
            x1s = xp.tile([128, NPAD], f16, tag="xs")
            aur = xp.tile([128, KT * WU8], u8, tag="aur")
            aer = xp.tile([128, KT * WE3], e3, tag="aer")
            cscs = xp.tile([128, 10], f32, tag="cscs")
            cscw = xp.tile([128, SP], f32, tag="cscw")
            w1s = xp.tile([128, 128], f16, tag="w1s")
            zl = xp.tile([128, 512], f16, tag="zl")
            warm = xp.tile([128, 1], f32, tag="warm")
            nc.scalar.dma_start(cscs[:], csc)
            if not w1_ones:
                nc.scalar.dma_start(w1s[:], w1b)
            nc.vector.memset(zl[:], 0.0)
            # pre-load the ACT tanh table so the layer-1 eviction doesn't
            # pay the table load on the critical path
            nc.scalar.activation(
                warm[:], zl[:, 0:1], mybir.ActivationFunctionType.Tanh
            )
            # broadcast csc[p, t] -> cscw[p, t*128+f] on the idle gpsimd
            # engine (a 5KB/partition csc DMA would delay the A stream)
            for t in range(10):
                nc.gpsimd.tensor_scalar_add(
                    cscw[:, t * 128:(t + 1) * 128],
                    zl[:].bitcast(f32)[:, 0:128],
                    cscs[:, t:t + 1],
                )

            agin = dr.tile([128, SP], f16)
            agout = dr.tile([NCORES * 128, SP], f16, addr_space="Shared")

            def fetch_range(b0, b1):
                # ae first: ktile t-order hits the e3m4 tiles (t<4) first
                dste = aer[:, b0 * WE3:b1 * WE3].rearrange(
                    "p (k j) -> p k j", k=b1 - b0
                )
                nc.sync.dma_start(
                    dste, ae[b0:b1].rearrange("k p j -> p k j")
                )
                dstu = aur[:, b0 * WU8:b1 * WU8].rearrange(
                    "p (k j) -> p k j", k=b1 - b0
                )
                nc.sync.dma_start(
                    dstu, au[b0:b1].rearrange("k p j -> p k j")
                )

            def fetch_groups(gi, halves=1):
                """DMA group gi of au + ae on the sync ring."""
                k0 = gi * GSIZE
                bounds = [k0 + (GSIZE * h) // halves for h in range(halves + 1)]
                for b0, b1 in zip(bounds, bounds[1:]):
                    if b0 != b1:
                        fetch_range(b0, b1)

            # rotation weighted by engine copy speed (ACT 0.83, DVE 1.04,
            # GpSimd 1.39 ns/row): ACT 2/5, DVE 2/5, Pool 1/5
            cast_engines = (nc.scalar, nc.vector, nc.scalar, nc.vector,
                            nc.gpsimd)

            def cast_u8(k, pool, tag, eng=None):
                """u8->f16 cast of AUR ktile k on a rotating engine."""
                st = pool.tile([128, WU8], f16, tag=tag)
                if eng is None:
                    eng = cast_engines[k % 5]
                src = aur[:, k * WU8:(k + 1) * WU8]
                if eng is nc.scalar:
                    nc.scalar.activation(
                        st[:], src, mybir.ActivationFunctionType.Copy
                    )
                else:
                    eng.tensor_copy(st[:], src)
                return st

            # ---- layer 1 (A-stationary; psum is [dst slot, feat]) ----
            # one PSUM tile per 2KiB bank so Tile scopes the eviction's RAW
            # dependency to that bank's stop=True matmul (a single [128,SP]
            # tile would serialize every eviction behind the LAST bank).
            # all PSUM tiles are full 2KiB banks: start=True resets the WHOLE
            # bank, so half-bank tiles sharing a bank would wipe each other
            p1 = [
                ps.tile([128, 512], f32, tag="acc1a", name="p1a"),
                ps.tile([128, 512], f32, tag="acc1b", name="p1b"),
                ps.tile([128, 512], f32, tag="acc1c", name="p1c"),
            ]
            pf = ps.tile([128, 512], f32, tag="pfill", name="pfill")
            # seed each layer-1 bank with one full-width start=True zero
            # matmul: the real matmuls write four 512B ranges per bank with
            # start=False (a per-range start=True would reset the whole bank
            # and erase the sibling ranges' first contributions)
            for pt in p1:
                nc.tensor.matmul(
                    pt[:], zl[:, 0:128], zl[:, 0:512],
                    start=True, stop=False,
                )
            # pre-filler: throwaway matmuls the scheduler hoists to t~1us.
            # They warm the tensor engine's p-state ramp clock and absorb
            # the first stream's ~4.5us DMA latency, so the real layer-1
            # matmuls start at full speed with a stream lead built up —
            # at DMA/PE parity a stall early in layer 1 is never recovered.
            for i in range(prefill):
                nc.tensor.matmul(
                    pf[:], zl[:, 0:128], zl[:, 0:512],
                    start=True, stop=(i == prefill - 1),
                )

            def l1_psum(t):
                b = min(t // 4, 2)
                pt = p1[b][:, (t - b * 4) * 128:(t - b * 4 + 1) * 128]
                if t == 9:
                    # the trimmed last dst tile has only 98 real columns;
                    # partitions 98-127 stay at the seeded zeros
                    pt = pt[0:S - 9 * 128]
                return pt

            def l1_lhsT(k, t, st):
                if t < NE3:
                    return aer[:, k * WE3 + t * 128:k * WE3 + (t + 1) * 128]
                o = (t - NE3) * 128
                return st[:, o:min(o + 128, WU8)]

            kt0 = KT - l1_tail  # start of the t-outer eviction tail
            for gi in range(NG):
                k0, k1 = gi * GSIZE, (gi + 1) * GSIZE
                if k0 == 0:
                    # startup order: first A half-group, tiny x0 chunk, rest
                    # — the first matmul's operands land as early as possible
                    fetch_range(0, 2)
                    nc.sync.dma_start(
                        x0s[:, 0:2 * 128], x0[:, 0:2 * 128]
                    )
                    fetch_range(2, 4)
                    nc.sync.dma_start(
                        x0s[:, 2 * 128:8 * 128], x0[:, 2 * 128:8 * 128]
                    )
                else:
                    if k0 % 8 == 0:
                        # x0 streamed in 8-ktile chunks (fewer DMAs -> less
                        # HWDGE/sem overhead on the shared rings)
                        nc.sync.dma_start(
                            x0s[:, k0 * 128:(k0 + 8) * 128],
                            x0[:, k0 * 128:(k0 + 8) * 128],
                        )
                    fetch_groups(gi)
                if k0 >= kt0:
                    continue
                for k in range(k0, k1):
                    st = cast_u8(k, s1pool, "s1")
                    rhs = x0s[:, k * 128:(k + 1) * 128]
                    for t in range(10):
                        nc.tensor.matmul(
                            l1_psum(t),
                            l1_lhsT(k, t, st),
                            rhs,
                            start=False, stop=False,
                        )

            # evict layer 1: x1 = tanh(cs_dst * psum) [* W1] on ACT (scale is
            # per partition, fused into the activation) into the AllGather
            # bounce, one agin DMA per PSUM bank.
            agin_sb = ev.tile([128, SP], f16, tag="agin")
            ob = ev.tile([128, SP], f32, tag="ob")
            obh = ev.tile([128, SP], f16, tag="obh")

            def evict_l1_bank(b0, b1):
                c0, c1 = b0 * 128, b1 * 128
                b = b0 // 4
                nc.vector.tensor_mul(
                    ob[:, c0:c1], p1[b][:, 0:c1 - c0], cscw[:, c0:c1]
                )
                nc.scalar.activation(
                    agin_sb[:, c0:c1], ob[:, c0:c1],
                    mybir.ActivationFunctionType.Tanh,
                )
                if not w1_ones:
                    for t in range(b0, b1):
                        nc.vector.tensor_mul(
                            agin_sb[:, t * 128:(t + 1) * 128],
                            agin_sb[:, t * 128:(t + 1) * 128], w1s[:],
                        )
                nc.sync.dma_start(agin[:, c0:c1], agin_sb[:, c0:c1])

            # final l1_tail ktiles bank-outer: each dst bank finishes
            # (stop=True) early and its tanh+agin DMA overlap the remaining
            # banks' matmuls; only the last (quarter-size) bank trails.
            # These casts gate the whole tail (its first t-row reads every
            # tail ktile), so pin them to the two fastest engines.
            tail_engs = (nc.scalar, nc.vector)
            sts = {
                k: cast_u8(k, s1pool, "s1", eng=tail_engs[k % 2])
                for k in range(kt0, KT)
            }
            for b0, b1 in ((0, 4), (4, 8), (8, 10)):
                for t in range(b0, b1):
                    for k in range(kt0, KT):
                        nc.tensor.matmul(
                            l1_psum(t),
                            l1_lhsT(k, t, sts[k]),
                            x0s[:, k * 128:(k + 1) * 128],
                            start=False,
                            stop=(k == KT - 1),
                        )
                evict_l1_bank(b0, b1)

            # layer-2 staging casts have no dependency on the collective:
            # pre-issue a ring's worth so the casters fill them during the
            # layer boundary while the AllGather is in flight.
            s2_pre = {k: cast_u8(k, s2pool, "s2") for k in range(s2bufs)}

            # Keep the PE busy across the layer boundary with throwaway
            # matmuls into a scratch PSUM bank: an idle gap here resets the
            # tensor engine's p-state ramp and the first ~3 us of layer 2
            # would run at half/quarter speed (costs ~9 us).  The filler
            # runs while the AllGather + x1 readback are in flight and is
            # sized to end just before rank 0's x1 lands.
            if filler:
                # anchor on the last eviction bank's tanh output so the
                # scheduler cannot hoist the filler before the boundary
                for i in range(filler):
                    nc.tensor.matmul(
                        pf[:], agin_sb[:, 0:128], zl[:, 0:512],
                        start=True, stop=(i == filler - 1),
                    )

            if nocc:
                nc.sync.dma_start(agout[0:128, :], agin[:])
            else:
                nc.gpsimd.collective_compute(
                    "AllGather",
                    mybir.AluOpType.bypass,
                    replica_groups=[list(range(NCORES))],
                    ins=[agin.opt()],
                    outs=[agout.opt()],
                )
            # agout rank blocks laid side by side in the free dim are exactly
            # layer-2's lhsT tiles in the same padded rank-block order A uses.
            # In the nocc twin only rows 0:128 of agout are written; read all
            # ranks from there so every readback carries the RAW dependency
            # on the collective stand-in (otherwise the sim fires them early
            # and they congest the HWDGE ring right at the layer boundary,
            # which the real program's post-collective readbacks never do).
            # Rank 0 is split so layer 2 restarts on its first ktiles as soon
            # as a quarter-shard lands.
            def ag_src(r):
                return 0 if nocc else r * 128

            nc.sync.dma_start(
                x1s[:, 0:256], agout[ag_src(0):ag_src(0) + 128, 0:256]
            )
            nc.sync.dma_start(
                x1s[:, 256:SP], agout[ag_src(0):ag_src(0) + 128, 256:SP]
            )
            for r in range(1, NCORES):
                nc.sync.dma_start(
                    x1s[:, r * SP:(r + 1) * SP],
                    agout[ag_src(r):ag_src(r) + 128, :],
                )

            # ---- layer 2 (X-stationary; psum is [feat, dst]) ----
            # again one PSUM tile per bank; chunk boundaries are bank-aligned
            # ((0,512) e3m4 direct, (512,512)+(1024,256) casted-f16)
            p2 = [
                ps.tile([128, 512], f32, tag="acc2a", name="p2a"),
                ps.tile([128, 512], f32, tag="acc2b", name="p2b"),
                ps.tile([128, 512], f32, tag="acc2c", name="p2c"),
            ]

            def l2_chunks(k, st):
                yield 0, p2[0][:, 0:512], aer[:, k * WE3:(k + 1) * WE3]
                yield 512, p2[1][:, 0:512], st[:, 0:512]
                yield 1024, p2[2][:, 0:226], st[:, 512:738]

            def l2_cast(k):
                if k in s2_pre:
                    return s2_pre[k]
                return cast_u8(k, s2pool, "s2")

            for k in range(KT - GSIZE):
                st = l2_cast(k)
                lhsT = x1s[:, k * 128:(k + 1) * 128]
                for c0, pt, rhs in l2_chunks(k, st):
                    nc.tensor.matmul(
                        pt[:], lhsT, rhs,
                        start=(k == 0), stop=False,
                    )
            # final group: bank-outer so each psum2 bank completes
            # (stop=True) early and its eviction overlaps the rest
            kf = KT - GSIZE
            sts = {kk: l2_cast(kk) for kk in range(kf, KT)}
            chunks = {kk: list(l2_chunks(kk, sts[kk])) for kk in range(kf, KT)}
            for ci in range(3):
                for kk in range(kf, KT):
                    c0, pt, rhs = chunks[kk][ci]
                    nc.tensor.matmul(
                        pt[:], x1s[:, kk * 128:(kk + 1) * 128], rhs,
                        start=False, stop=(kk == KT - 1),
                    )
                c0, pt, _ = chunks[kf][ci]
                cn = 512 if ci < 2 else 226
                nc.vector.tensor_copy(obh[:, c0:c0 + cn], pt[:])
                nc.sync.dma_start(out[:, c0:c0 + cn], obh[:, c0:c0 + cn])

    nc.compile()
    return nc


def get_program(nocc=False, w1_ones=True, **kw):
    key = (nocc, w1_ones, tuple(sorted(kw.items())))
    if key not in _PROG_CACHE:
        _PROG_CACHE[key] = _build_program(nocc, w1_ones, **kw)
    return _PROG_CACHE[key]


def _node_perm():
    """Padded rank-block src ordering: slot i <-> (rank r = i//1280,
    local q = i%1280); global node r*1250+q for q<1250, else pad."""
    i2 = np.arange(NPAD)
    r2 = i2 // SP
    loc = i2 % SP
    node = r2 * S + loc
    valid = loc < S
    return np.where(valid, node, 0), valid


def build_in_maps(x, src, dst, vals, W):
    """Host-side prep: dense A^T shard (e3m4 + u8 per-column quantized)."""
    import scipy.sparse as sp

    x = np.asarray(x, np.float32)
    src = np.asarray(src, np.int64)
    dst = np.asarray(dst, np.int64)
    vals = np.asarray(vals, np.float32)
    W = np.asarray(W, np.float32)

    # A[dst, src] = sum of vals  ->  we build AT[src, dst]
    AT = sp.coo_matrix((vals, (src, dst)), shape=(N, N)).toarray()

    node2, valid2 = _node_perm()

    xw = x * W[0][None, :]
    x0p = np.zeros((NPAD, D), np.float32)
    x0p[valid2] = xw[node2[valid2]]
    x0h = np.ascontiguousarray(
        x0p.reshape(KT, 128, D).transpose(1, 0, 2).reshape(128, KT * D)
    ).astype(np.float16)

    w1brow = np.ascontiguousarray(
        np.tile(W[1][None, :], (128, 1))
    ).astype(np.float16)

    in_maps = []
    steps = []
    for c in range(NCORES):
        ATc = AT[:, c * S:(c + 1) * S]  # [N, S] float32
        colmax = np.maximum(ATc.max(axis=0), 1e-9)
        # permute + pad src rows once, in f32
        Ap = np.zeros((NPAD, SP), np.float32)
        Ap[valid2, :S] = ATc[node2[valid2]]
        # dequant scale per padded slot
        scale_pad = np.zeros(SP, np.float32)
        cm_pad = np.zeros(SP, np.float32)
        cm_pad[:S] = colmax
        cm_pad[S:] = 1.0
        # e3m4 tiles: slots [0, WE3)
        sc_e3 = E3_TARGET / np.maximum(cm_pad[:WE3], 1e-9)
        Ae = (Ap[:, :WE3] * sc_e3[None, :]).astype(ml_dtypes.float8_e3m4)
        scale_pad[:WE3] = 1.0 / sc_e3
        # u8 tiles: slots [WE3, S) — the pad columns [S, SP) are all-zero
        # and never touched on device
        step = cm_pad[WE3:S] / 255.0
        Au = np.clip(
            np.rint(Ap[:, WE3:S] * (1.0 / step)[None, :]), 0, 255
        ).astype(np.uint8)
        scale_pad[WE3:S] = step
        steps.append(scale_pad)
        # csc[p, t] = dequant scale of dst slot t*128+p
        csc_tile = np.ascontiguousarray(
            scale_pad.reshape(10, 128).T
        ).astype(np.float32)
        in_maps.append(
            {
                "au": np.ascontiguousarray(Au.reshape(KT, 128, WU8)),
                "ae": np.ascontiguousarray(Ae.reshape(KT, 128, WE3)),
                "x0": x0h,
                "csc": csc_tile,
                "w1b": w1brow,
            }
        )
    return in_maps, steps


def assemble_output(results, steps):
    outs = []
    for c in range(NCORES):
        ot = np.asarray(results[c]["out"], np.float32)  # [128, SP] feat-major
        ot = ot * steps[c][None, :]  # per-dst dequant (layer-2)
        outs.append(ot[:, :S].T)
    return np.ascontiguousarray(np.concatenate(outs, axis=0))


def kernel(x, src, dst, vals, W):
    from concourse import bass_utils

    w1_ones = bool(np.all(np.asarray(W)[1] == 1.0))
    nc = get_program(w1_ones=w1_ones)
    in_maps, steps = build_in_maps(x, src, dst, vals, W)
    # The axon terminal can wedge when a different program was loaded
    # earlier in its lifetime; after the crash the terminal restarts and a
    # retry succeeds.  Back off progressively to ride out the restart.
    import time as _time

    last_err = None
    for sleep_s in (10.0, 30.0, 60.0, 0.0):
        try:
            res = bass_utils.run_bass_kernel_spmd(
                nc, in_maps, core_ids=list(range(NCORES))
            )
            return assemble_output(res.results, steps)
        except Exception as e:  # noqa: BLE001
            last_err = e
            _time.sleep(sleep_s)
    raise last_err


# revision 85
# speedup vs baseline: 1.3921x; 1.0016x over previous
"""GCN diag-encoder (2-layer SpMM) on 8 Trainium2 NeuronCores.

Strategy: the sparse adjacency (640K edges over 10K nodes, ~0.64% dense) is
materialized as a dense A^T on the host; each per-layer
  out[dst] = sum_e vals[e] * x[src[e]]        (segment-sum SpMM)
becomes dense TensorEngine matmuls.  Each core owns a 1250-wide dst slice of
A^T (padded to 1280 = 10 tiles of 128 dst slots).

A^T is kept FULLY RESIDENT in SBUF in one-byte-per-element form, streamed
from HBM exactly once (~13 MB, ~36 us of DMA), so neither layer re-streams
it.  Two per-column quantizations split the dst tiles:
  - tiles 0-3  (slots    0- 511): fp8 e3m4, consumed by the PE directly
    (both as layer-1 lhsT and layer-2 moving operand; fp8e3 runs at the
    same 1 row/cycle as f16 in the cost model),
  - tiles 4-9  (slots 512-1249; the 30 pad slots are dropped): uint8
    (255-level, ~4x lower quantization error than e3m4), cast u8->f16 into
    small staging rings by the otherwise-idle Activation/Vector/GpSimd
    engines right before use.
The mix keeps the end-to-end relative error ~1.4e-2 (vs 2.2e-2 for pure
e3m4) while the DMA pool only ever moves one byte per A element.  The
baseline instead streamed A u8 with an inline u8->f16 cast DMA — which the
cost model charges at the 2-byte WRITE side — plus an f16 re-stream for
layer 2, making it DMA-bound (121 us DMA vs 92 us PE).  Here DMA drops to
~50 us and the kernel is PE-bound (~84 us of matmul rows).  Throwaway
"filler" matmuls keep the tensor engine busy at program start (absorbing
the stream's pipeline-fill latency) and across the AllGather boundary —
an idle PE gap resets the cost model's p-state ramp and the next ~3us of
matmuls would run at half speed.

Layer 1 runs A-stationary — matmul(out=psum[dst,feat], lhsT=AT_tile[src,dst],
rhs=x_tile[src,feat]) — so the layer-1 output is already node-major: the
eviction is a fused tanh+dequant-scale pass on the scalar engine (scale is
per dst node = per partition) straight into the AllGather bounce.  PSUM
accumulation groups are per 2KiB bank while layer 1 writes four 512B ranges
per bank, so each bank is seeded by one full-width start=True zero matmul.
Layer 2 (PE-bound) runs X-stationary — matmul(out=psum[feat,dst],
lhsT=x1_tile[src,feat], rhs=AT_chunk[src,dst]); its chunks (0,512)=e3m4,
(512,512)+(1024,226)=casted-f16 are PSUM-bank aligned.  The layer-2
dequant scale (per dst = per free element) and the final transpose are
applied on the host; the output travels as f16 (values ~±26 against a
2e-2 relative tolerance).

Src nodes use a padded rank-block ordering (rank r owns slots
r*1280..r*1280+1279) so layer 2's AllGathered activations line up with the
SAME A arrangement layer 1 uses.  The layer-2 u8->f16 staging casts have no
dependency on the AllGather, so they run ahead during the layer boundary
and the PE restarts on rank 0's x1 chunk as soon as it lands.  W0 is folded
into x on the host; W1 is skipped on device when it is all-ones (torch
init), else applied via a broadcast multiply.
"""

import numpy as np
import ml_dtypes

N = 10000          # nodes
D = 128            # feature dim
NCORES = 8
S = 1250           # dst nodes per core
SP = 1280          # padded dst per core (10 tiles of 128)
KT = 80            # contraction k-tiles (padded src rows = 10240)
NPAD = KT * 128    # 10240
GSIZE = 4          # k-tiles per DMA/cast group
NG = KT // GSIZE   # 20 groups
NE3 = 4            # leading dst tiles stored as fp8 e3m4 (slots 0-511)
WE3 = NE3 * 128            # 512
WU8 = S - WE3              # 738 (real dst only; pad slots 1250-1279 dropped)
E3_TARGET = 7.8    # colmax maps to ~7.8 so values sit in e3m4's sweet spot
BF16 = ml_dtypes.bfloat16

_PROG_CACHE = {}


def _build_program(nocc=False, w1_ones=True, s1bufs=12, s2bufs=16,
                   l1_tail=4, filler=40, prefill=12):
    import concourse.bacc as bacc
    import concourse.mybir as mybir
    from concourse import tile

    f32 = mybir.dt.float32
    f16 = mybir.dt.float16
    e3 = mybir.dt.float8e3
    u8 = mybir.dt.uint8

    nc = bacc.Bacc(
        "TRN2",
        target_bir_lowering=False,
        debug=False,
        enable_asserts=False,
        num_devices=1 if nocc else NCORES,
    )

    au = nc.dram_tensor("au", [KT, 128, WU8], u8, kind="ExternalInput").ap()
    ae = nc.dram_tensor("ae", [KT, 128, WE3], e3, kind="ExternalInput").ap()
    x0 = nc.dram_tensor("x0", [128, NPAD], f16, kind="ExternalInput").ap()
    # per-dst-node dequant scales, [slot p, tile t] layout
    csc = nc.dram_tensor("csc", [128, 10], f32, kind="ExternalInput").ap()
    # broadcast W1 row (only read when not w1_ones)
    w1b = nc.dram_tensor("w1b", [128, 128], f16, kind="ExternalInput").ap()
    # f16 output (values ~±26, rel tolerance 2e-2 — f16 rounding is noise);
    # written straight from PSUM, halving the final eviction DMA traffic
    out = nc.dram_tensor("out", [128, SP], f16, kind="ExternalOutput").ap()

    with tile.TileContext(nc) as tc:
        with (
            tc.tile_pool(name="xp", bufs=1) as xp,
            tc.tile_pool(name="s1", bufs=s1bufs) as s1pool,
            tc.tile_pool(name="s2", bufs=s2bufs) as s2pool,
            tc.tile_pool(name="ev", bufs=1) as ev,
            tc.tile_pool(name="ps", bufs=1, space="PSUM") as ps,
            tc.tile_pool(name="dr", bufs=1, space="DRAM") as dr,
        ):
            # x0 is dead once layer 1 finishes; share one slot for both
            x0s = xp.tile([128, NPAD], f16, tag="xs")
            x1s = xp.tile([128, NPAD], f16, tag="xs")
            aur = xp.tile([128, KT * WU8], u8, tag="aur")
            aer = xp.tile([128, KT * WE3], e3, tag="aer")
            cscs = xp.tile([128, 10], f32, tag="cscs")
            cscw = xp.tile([128, SP], f32, tag="cscw")
            w1s = xp.tile([128, 128], f16, tag="w1s")
            zl = xp.tile([128, 512], f16, tag="zl")
            warm = xp.tile([128, 1], f32, tag="warm")
            nc.scalar.dma_start(cscs[:], csc)
            if not w1_ones:
                nc.scalar.dma_start(w1s[:], w1b)
            nc.vector.memset(zl[:], 0.0)
            # pre-load the ACT tanh table so the layer-1 eviction doesn't
            # pay the table load on the critical path
            nc.scalar.activation(
                warm[:], zl[:, 0:1], mybir.ActivationFunctionType.Tanh
            )
            # broadcast csc[p, t] -> cscw[p, t*128+f] on the idle gpsimd
            # engine (a 5KB/partition csc DMA would delay the A stream)
            for t in range(10):
                nc.gpsimd.tensor_scalar_add(
                    cscw[:, t * 128:(t + 1) * 128],
                    zl[:].bitcast(f32)[:, 0:128],
                    cscs[:, t:t + 1],
                )

            agin = dr.tile([128, SP], f16)
            agout = dr.tile([NCORES * 128, SP], f16, addr_space="Shared")

            def fetch_range(b0, b1):
                # ae first: ktile t-order hits the e3m4 tiles (t<4) first
                dste = aer[:, b0 * WE3:b1 * WE3].rearrange(
                    "p (k j) -> p k j", k=b1 - b0
                )
                nc.sync.dma_start(
                    dste, ae[b0:b1].rearrange("k p j -> p k j")
                )
                dstu = aur[:, b0 * WU8:b1 * WU8].rearrange(
                    "p (k j) -> p k j", k=b1 - b0
                )
                nc.sync.dma_start(
                    dstu, au[b0:b1].rearrange("k p j -> p k j")
                )

            def fetch_groups(gi, halves=1):
                """DMA group gi of au + ae on the sync ring."""
                k0 = gi * GSIZE
                bounds = [k0 + (GSIZE * h) // halves for h in range(halves + 1)]
                for b0, b1 in zip(bounds, bounds[1:]):
                    if b0 != b1:
                        fetch_range(b0, b1)

            # rotation weighted by measured cast cost (DVE 445ns, ACT 800ns,
            # GpSimd 1120ns per [128,738] chunk): DVE 3/5, ACT 1/5, Pool 1/5
            cast_engines = (nc.vector, nc.scalar, nc.vector, nc.gpsimd,
                            nc.vector)

            def cast_u8(k, pool, tag, eng=None):
                """u8->f16 cast of AUR ktile k on a rotating engine."""
                st = pool.tile([128, WU8], f16, tag=tag)
                if eng is None:
                    eng = cast_engines[k % 5]
                src = aur[:, k * WU8:(k + 1) * WU8]
                if eng is nc.scalar:
                    nc.scalar.activation(
                        st[:], src, mybir.ActivationFunctionType.Copy
                    )
                else:
                    eng.tensor_copy(st[:], src)
                return st

            # ---- layer 1 (A-stationary; psum is [dst slot, feat]) ----
            # one PSUM tile per 2KiB bank so Tile scopes the eviction's RAW
            # dependency to that bank's stop=True matmul (a single [128,SP]
            # tile would serialize every eviction behind the LAST bank).
            # all PSUM tiles are full 2KiB banks: start=True resets the WHOLE
            # bank, so half-bank tiles sharing a bank would wipe each other
            p1 = [
                ps.tile([128, 512], f32, tag="acc1a", name="p1a"),
                ps.tile([128, 512], f32, tag="acc1b", name="p1b"),
                ps.tile([128, 512], f32, tag="acc1c", name="p1c"),
            ]
            pf = ps.tile([128, 512], f32, tag="pfill", name="pfill")
            # seed each layer-1 bank with one full-width start=True zero
            # matmul: the real matmuls write four 512B ranges per bank with
            # start=False (a per-range start=True would reset the whole bank
            # and erase the sibling ranges' first contributions)
            for pt in p1:
                nc.tensor.matmul(
                    pt[:], zl[:, 0:128], zl[:, 0:512],
                    start=True, stop=False,
                )
            # pre-filler: throwaway matmuls the scheduler hoists to t~1us.
            # They warm the tensor engine's p-state ramp clock and absorb
            # the first stream's ~4.5us DMA latency, so the real layer-1
            # matmuls start at full speed with a stream lead built up —
            # at DMA/PE parity a stall early in layer 1 is never recovered.
            for i in range(prefill):
                nc.tensor.matmul(
                    pf[:], zl[:, 0:128], zl[:, 0:512],
                    start=True, stop=(i == prefill - 1),
                )

            def l1_psum(t):
                b = min(t // 4, 2)
                pt = p1[b][:, (t - b * 4) * 128:(t - b * 4 + 1) * 128]
                if t == 9:
                    # the trimmed last dst tile has only 98 real columns;
                    # partitions 98-127 stay at the seeded zeros
                    pt = pt[0:S - 9 * 128]
                return pt

            def l1_lhsT(k, t, st):
                if t < NE3:
                    return aer[:, k * WE3 + t * 128:k * WE3 + (t + 1) * 128]
                o = (t - NE3) * 128
                return st[:, o:min(o + 128, WU8)]

            kt0 = KT - l1_tail  # start of the t-outer eviction tail
            for gi in range(NG):
                k0, k1 = gi * GSIZE, (gi + 1) * GSIZE
                if k0 == 0:
                    # startup order: first A half-group, tiny x0 chunk, rest
                    # — the first matmul's operands land as early as possible
                    fetch_range(0, 2)
                    nc.sync.dma_start(
                        x0s[:, 0:2 * 128], x0[:, 0:2 * 128]
                    )
                    fetch_range(2, 4)
                    nc.sync.dma_start(
                        x0s[:, 2 * 128:8 * 128], x0[:, 2 * 128:8 * 128]
                    )
                else:
                    if k0 % 8 == 0:
                        # x0 streamed in 8-ktile chunks (fewer DMAs -> less
                        # HWDGE/sem overhead on the shared rings)
                        nc.sync.dma_start(
                            x0s[:, k0 * 128:(k0 + 8) * 128],
                            x0[:, k0 * 128:(k0 + 8) * 128],
                        )
                    # the tail group's casts gate the whole t-outer tail:
                    # fetch it in halves so they start ~1us earlier
                    fetch_groups(gi, halves=2 if k0 >= kt0 else 1)
                if k0 >= kt0:
                    continue
                for k in range(k0, k1):
                    st = cast_u8(k, s1pool, "s1")
                    rhs = x0s[:, k * 128:(k + 1) * 128]
                    for t in range(10):
                        nc.tensor.matmul(
                            l1_psum(t),
                            l1_lhsT(k, t, st),
                            rhs,
                            start=False, stop=False,
                        )

            # evict layer 1: x1 = tanh(cs_dst * psum) [* W1] on ACT (scale is
            # per partition, fused into the activation) into the AllGather
            # bounce, one agin DMA per PSUM bank.
            agin_sb = ev.tile([128, SP], f16, tag="agin")
            ob = ev.tile([128, SP], f32, tag="ob")
            obh = ev.tile([128, SP], f16, tag="obh")

            def evict_l1_bank(b0, b1):
                c0, c1 = b0 * 128, b1 * 128
                b = b0 // 4
                nc.vector.tensor_mul(
                    ob[:, c0:c1], p1[b][:, 0:c1 - c0], cscw[:, c0:c1]
                )
                nc.scalar.activation(
                    agin_sb[:, c0:c1], ob[:, c0:c1],
                    mybir.ActivationFunctionType.Tanh,
                )
                if not w1_ones:
                    for t in range(b0, b1):
                        nc.vector.tensor_mul(
                            agin_sb[:, t * 128:(t + 1) * 128],
                            agin_sb[:, t * 128:(t + 1) * 128], w1s[:],
                        )
                nc.sync.dma_start(agin[:, c0:c1], agin_sb[:, c0:c1])

            # final l1_tail ktiles bank-outer: each dst bank finishes
            # (stop=True) early and its tanh+agin DMA overlap the remaining
            # banks' matmuls; only the last (quarter-size) bank trails.
            # These casts gate the whole tail (its first t-row reads every
            # tail ktile), so pin them to the two fastest engines.
            tail_engs = (nc.vector, nc.scalar, nc.vector, nc.vector)
            sts = {
                k: cast_u8(k, s1pool, "s1", eng=tail_engs[k % 4])
                for k in range(kt0, KT)
            }
            for b0, b1 in ((0, 4), (4, 8), (8, 10)):
                for t in range(b0, b1):
                    for k in range(kt0, KT):
                        nc.tensor.matmul(
                            l1_psum(t),
                            l1_lhsT(k, t, sts[k]),
                            x0s[:, k * 128:(k + 1) * 128],
                            start=False,
                            stop=(k == KT - 1),
                        )
                evict_l1_bank(b0, b1)

            # layer-2 staging casts have no dependency on the collective:
            # pre-issue a ring's worth so the casters fill them during the
            # layer boundary while the AllGather is in flight.
            s2_pre = {k: cast_u8(k, s2pool, "s2") for k in range(s2bufs)}

            # Keep the PE busy across the layer boundary with throwaway
            # matmuls into a scratch PSUM bank: an idle gap here resets the
            # tensor engine's p-state ramp and the first ~3 us of layer 2
            # would run at half/quarter speed (costs ~9 us).  The filler
            # runs while the AllGather + x1 readback are in flight and is
            # sized to end just before rank 0's x1 lands.
            if filler:
                # anchor on the last eviction bank's tanh output so the
                # scheduler cannot hoist the filler before the boundary
                for i in range(filler):
                    nc.tensor.matmul(
                        pf[:], agin_sb[:, 0:128], zl[:, 0:512],
                        start=True, stop=(i == filler - 1),
                    )

            if nocc:
                nc.sync.dma_start(agout[0:128, :], agin[:])
            else:
                nc.gpsimd.collective_compute(
                    "AllGather",
                    mybir.AluOpType.bypass,
                    replica_groups=[list(range(NCORES))],
                    ins=[agin.opt()],
                    outs=[agout.opt()],
                )
            # agout rank blocks laid side by side in the free dim are exactly
            # layer-2's lhsT tiles in the same padded rank-block order A uses.
            # In the nocc twin only rows 0:128 of agout are written; read all
            # ranks from there so every readback carries the RAW dependency
            # on the collective stand-in (otherwise the sim fires them early
            # and they congest the HWDGE ring right at the layer boundary,
            # which the real program's post-collective readbacks never do).
            # Rank 0 is split so layer 2 restarts on its first ktiles as soon
            # as a quarter-shard lands.
            def ag_src(r):
                return 0 if nocc else r * 128

            nc.sync.dma_start(
                x1s[:, 0:256], agout[ag_src(0):ag_src(0) + 128, 0:256]
            )
            nc.sync.dma_start(
                x1s[:, 256:SP], agout[ag_src(0):ag_src(0) + 128, 256:SP]
            )
            for r in range(1, NCORES):
                nc.sync.dma_start(
                    x1s[:, r * SP:(r + 1) * SP],
                    agout[ag_src(r):ag_src(r) + 128, :],
                )

            # ---- layer 2 (X-stationary; psum is [feat, dst]) ----
            # again one PSUM tile per bank; chunk boundaries are bank-aligned
            # ((0,512) e3m4 direct, (512,512)+(1024,256) casted-f16)
            p2 = [
                ps.tile([128, 512], f32, tag="acc2a", name="p2a"),
                ps.tile([128, 512], f32, tag="acc2b", name="p2b"),
                ps.tile([128, 512], f32, tag="acc2c", name="p2c"),
            ]

            def l2_chunks(k, st):
                yield 0, p2[0][:, 0:512], aer[:, k * WE3:(k + 1) * WE3]
                yield 512, p2[1][:, 0:512], st[:, 0:512]
                yield 1024, p2[2][:, 0:226], st[:, 512:738]

            def l2_cast(k):
                if k in s2_pre:
                    return s2_pre[k]
                return cast_u8(k, s2pool, "s2")

            for k in range(KT - GSIZE):
                st = l2_cast(k)
                lhsT = x1s[:, k * 128:(k + 1) * 128]
                for c0, pt, rhs in l2_chunks(k, st):
                    nc.tensor.matmul(
                        pt[:], lhsT, rhs,
                        start=(k == 0), stop=False,
                    )
            # final group: bank-outer so each psum2 bank completes
            # (stop=True) early and its eviction overlaps the rest
            kf = KT - GSIZE
            sts = {kk: l2_cast(kk) for kk in range(kf, KT)}
            chunks = {kk: list(l2_chunks(kk, sts[kk])) for kk in range(kf, KT)}
            for ci in range(3):
                for kk in range(kf, KT):
                    c0, pt, rhs = chunks[kk][ci]
                    nc.tensor.matmul(
                        pt[:], x1s[:, kk * 128:(kk + 1) * 128], rhs,
                        start=False, stop=(kk == KT - 1),
                    )
                c0, pt, _ = chunks[kf][ci]
                cn = 512 if ci < 2 else 226
                nc.vector.tensor_copy(obh[:, c0:c0 + cn], pt[:])
                nc.sync.dma_start(out[:, c0:c0 + cn], obh[:, c0:c0 + cn])

    nc.compile()
    return nc


def get_program(nocc=False, w1_ones=True, **kw):
    key = (nocc, w1_ones, tuple(sorted(kw.items())))
    if key not in _PROG_CACHE:
        _PROG_CACHE[key] = _build_program(nocc, w1_ones, **kw)
    return _PROG_CACHE[key]


def _node_perm():
    """Padded rank-block src ordering: slot i <-> (rank r = i//1280,
    local q = i%1280); global node r*1250+q for q<1250, else pad."""
    i2 = np.arange(NPAD)
    r2 = i2 // SP
    loc = i2 % SP
    node = r2 * S + loc
    valid = loc < S
    return np.where(valid, node, 0), valid


def build_in_maps(x, src, dst, vals, W):
    """Host-side prep: dense A^T shard (e3m4 + u8 per-column quantized)."""
    import scipy.sparse as sp

    x = np.asarray(x, np.float32)
    src = np.asarray(src, np.int64)
    dst = np.asarray(dst, np.int64)
    vals = np.asarray(vals, np.float32)
    W = np.asarray(W, np.float32)

    # A[dst, src] = sum of vals  ->  we build AT[src, dst]
    AT = sp.coo_matrix((vals, (src, dst)), shape=(N, N)).toarray()

    node2, valid2 = _node_perm()

    xw = x * W[0][None, :]
    x0p = np.zeros((NPAD, D), np.float32)
    x0p[valid2] = xw[node2[valid2]]
    x0h = np.ascontiguousarray(
        x0p.reshape(KT, 128, D).transpose(1, 0, 2).reshape(128, KT * D)
    ).astype(np.float16)

    w1brow = np.ascontiguousarray(
        np.tile(W[1][None, :], (128, 1))
    ).astype(np.float16)

    in_maps = []
    steps = []
    for c in range(NCORES):
        ATc = AT[:, c * S:(c + 1) * S]  # [N, S] float32
        colmax = np.maximum(ATc.max(axis=0), 1e-9)
        # permute + pad src rows once, in f32
        Ap = np.zeros((NPAD, SP), np.float32)
        Ap[valid2, :S] = ATc[node2[valid2]]
        # dequant scale per padded slot
        scale_pad = np.zeros(SP, np.float32)
        cm_pad = np.zeros(SP, np.float32)
        cm_pad[:S] = colmax
        cm_pad[S:] = 1.0
        # e3m4 tiles: slots [0, WE3)
        sc_e3 = E3_TARGET / np.maximum(cm_pad[:WE3], 1e-9)
        Ae = (Ap[:, :WE3] * sc_e3[None, :]).astype(ml_dtypes.float8_e3m4)
        scale_pad[:WE3] = 1.0 / sc_e3
        # u8 tiles: slots [WE3, S) — the pad columns [S, SP) are all-zero
        # and never touched on device
        step = cm_pad[WE3:S] / 255.0
        Au = np.clip(
            np.rint(Ap[:, WE3:S] * (1.0 / step)[None, :]), 0, 255
        ).astype(np.uint8)
        scale_pad[WE3:S] = step
        steps.append(scale_pad)
        # csc[p, t] = dequant scale of dst slot t*128+p
        csc_tile = np.ascontiguousarray(
            scale_pad.reshape(10, 128).T
        ).astype(np.float32)
        in_maps.append(
            {
                "au": np.ascontiguousarray(Au.reshape(KT, 128, WU8)),
                "ae": np.ascontiguousarray(Ae.reshape(KT, 128, WE3)),
                "x0": x0h,
                "csc": csc_tile,
                "w1b": w1brow,
            }
        )
    return in_maps, steps


def assemble_output(results, steps):
    outs = []
    for c in range(NCORES):
        ot = np.asarray(results[c]["out"], np.float32)  # [128, SP] feat-major
        ot = ot * steps[c][None, :]  # per-dst dequant (layer-2)
        outs.append(ot[:, :S].T)
    return np.ascontiguousarray(np.concatenate(outs, axis=0))


def kernel(x, src, dst, vals, W):
    from concourse import bass_utils

    w1_ones = bool(np.all(np.asarray(W)[1] == 1.0))
    nc = get_program(w1_ones=w1_ones)
    in_maps, steps = build_in_maps(x, src, dst, vals, W)
    # The axon terminal can wedge when a different program was loaded
    # earlier in its lifetime; after the crash the terminal restarts and a
    # retry succeeds.  Back off progressively to ride out the restart.
    import time as _time

    last_err = None
    for sleep_s in (10.0, 30.0, 60.0, 0.0):
        try:
            res = bass_utils.run_bass_kernel_spmd(
                nc, in_maps, core_ids=list(range(NCORES))
            )
            return assemble_output(res.results, steps)
        except Exception as e:  # noqa: BLE001
            last_err = e
            _time.sleep(sleep_s)
    raise last_err


# revision 109
# speedup vs baseline: 1.4171x; 1.0179x over previous
"""GCN diag-encoder (2-layer SpMM) on 8 Trainium2 NeuronCores.

Strategy: the sparse adjacency (640K edges over 10K nodes, ~0.64% dense) is
materialized as a dense A^T on the host; each per-layer
  out[dst] = sum_e vals[e] * x[src[e]]        (segment-sum SpMM)
becomes dense TensorEngine matmuls.  Each core owns a 1250-wide dst slice of
A^T (padded to 1280 = 10 tiles of 128 dst slots).

A^T is kept FULLY RESIDENT in SBUF in one-byte-per-element form, streamed
from HBM exactly once (~13 MB, ~36 us of DMA), so neither layer re-streams
it.  Two per-column quantizations split the dst tiles:
  - tiles 0-3  (slots    0- 511): fp8 e3m4, consumed by the PE directly
    (both as layer-1 lhsT and layer-2 moving operand; fp8e3 runs at the
    same 1 row/cycle as f16 in the cost model),
  - tiles 4-9  (slots 512-1249; the 30 pad slots are dropped): uint8
    (255-level, ~4x lower quantization error than e3m4), cast u8->f16 into
    small staging rings by the otherwise-idle Activation/Vector/GpSimd
    engines right before use.
The mix keeps the end-to-end relative error ~1.4e-2 (vs 2.2e-2 for pure
e3m4) while the DMA pool only ever moves one byte per A element.  The
baseline instead streamed A u8 with an inline u8->f16 cast DMA — which the
cost model charges at the 2-byte WRITE side — plus an f16 re-stream for
layer 2, making it DMA-bound (121 us DMA vs 92 us PE).  Here DMA drops to
~50 us and the kernel is PE-bound (~84 us of matmul rows).  Throwaway
"filler" matmuls keep the tensor engine busy at program start (absorbing
the stream's pipeline-fill latency) and across the AllGather boundary —
an idle PE gap resets the cost model's p-state ramp and the next ~3us of
matmuls would run at half speed.

Layer 1 runs A-stationary — matmul(out=psum[dst,feat], lhsT=AT_tile[src,dst],
rhs=x_tile[src,feat]) — so the layer-1 output is already node-major: the
eviction is a fused tanh+dequant-scale pass on the scalar engine (scale is
per dst node = per partition) straight into the AllGather bounce.  PSUM
accumulation groups are per 2KiB bank while layer 1 writes four 512B ranges
per bank, so each bank is seeded by one full-width start=True zero matmul.
Layer 2 (PE-bound) runs X-stationary — matmul(out=psum[feat,dst],
lhsT=x1_tile[src,feat], rhs=AT_chunk[src,dst]); its chunks (0,512)=e3m4,
(512,512)+(1024,226)=casted-f16 are PSUM-bank aligned.  The layer-2
dequant scale (per dst = per free element) and the final transpose are
applied on the host; the output travels as f16 (values ~±26 against a
2e-2 relative tolerance).

Src nodes use a padded rank-block ordering (rank r owns slots
r*1280..r*1280+1279) so layer 2's AllGathered activations line up with the
SAME A arrangement layer 1 uses.  The layer-2 u8->f16 staging casts have no
dependency on the AllGather, so they run ahead during the layer boundary
and the PE restarts on rank 0's x1 chunk as soon as it lands.  W0 is folded
into x on the host; W1 is skipped on device when it is all-ones (torch
init), else applied via a broadcast multiply.
"""

import numpy as np
import ml_dtypes

N = 10000          # nodes
D = 128            # feature dim
NCORES = 8
S = 1250           # dst nodes per core
SP = 1280          # padded dst per core (10 tiles of 128)
KT = 80            # contraction k-tiles (padded src rows = 10240)
NPAD = KT * 128    # 10240
GSIZE = 4          # k-tiles per DMA/cast group
NG = KT // GSIZE   # 20 groups
NE3 = 4            # leading dst tiles stored as fp8 e3m4 (slots 0-511)
WE3 = NE3 * 128            # 512
WU8 = S - WE3              # 738 (real dst only; pad slots 1250-1279 dropped)
E3_TARGET = 7.8    # colmax maps to ~7.8 so values sit in e3m4's sweet spot
BF16 = ml_dtypes.bfloat16

_PROG_CACHE = {}


def _build_program(nocc=False, w1_ones=True, s1bufs=12, s2bufs=16,
                   l1_tail=8, filler=33, prefill=12):
    import concourse.bacc as bacc
    import concourse.mybir as mybir
    from concourse import tile

    f32 = mybir.dt.float32
    f16 = mybir.dt.float16
    e3 = mybir.dt.float8e3
    u8 = mybir.dt.uint8

    nc = bacc.Bacc(
        "TRN2",
        target_bir_lowering=False,
        debug=False,
        enable_asserts=False,
        num_devices=1 if nocc else NCORES,
    )

    au = nc.dram_tensor("au", [KT, 128, WU8], u8, kind="ExternalInput").ap()
    ae = nc.dram_tensor("ae", [KT, 128, WE3], e3, kind="ExternalInput").ap()
    x0 = nc.dram_tensor("x0", [128, NPAD], f16, kind="ExternalInput").ap()
    # per-dst-node dequant scales, [slot p, tile t] layout
    csc = nc.dram_tensor("csc", [128, 10], f32, kind="ExternalInput").ap()
    # broadcast W1 row (only read when not w1_ones)
    w1b = nc.dram_tensor("w1b", [128, 128], f16, kind="ExternalInput").ap()
    # f16 output (values ~±26, rel tolerance 2e-2 — f16 rounding is noise);
    # written straight from PSUM, halving the final eviction DMA traffic
    out = nc.dram_tensor("out", [128, SP], f16, kind="ExternalOutput").ap()

    with tile.TileContext(nc) as tc:
        with (
            tc.tile_pool(name="xp", bufs=1) as xp,
            tc.tile_pool(name="s1", bufs=s1bufs) as s1pool,
            tc.tile_pool(name="s2", bufs=s2bufs) as s2pool,
            tc.tile_pool(name="ev", bufs=1) as ev,
            tc.tile_pool(name="ps", bufs=1, space="PSUM") as ps,
            tc.tile_pool(name="dr", bufs=1, space="DRAM") as dr,
        ):
            # x0 is dead once layer 1 finishes; share one slot for both
            x0s = xp.tile([128, NPAD], f16, tag="xs")
            x1s = xp.tile([128, NPAD], f16, tag="xs")
            aur = xp.tile([128, KT * WU8], u8, tag="aur")
            aer = xp.tile([128, KT * WE3], e3, tag="aer")
            cscs = xp.tile([128, 10], f32, tag="cscs")
            cscw = xp.tile([128, SP], f32, tag="cscw")
            w1s = xp.tile([128, 128], f16, tag="w1s")
            zl = xp.tile([128, 512], f16, tag="zl")
            warm = xp.tile([128, 1], f32, tag="warm")
            nc.scalar.dma_start(cscs[:], csc)
            if not w1_ones:
                nc.scalar.dma_start(w1s[:], w1b)
            nc.vector.memset(zl[:], 0.0)
            # pre-load the ACT tanh table so the layer-1 eviction doesn't
            # pay the table load on the critical path
            nc.scalar.activation(
                warm[:], zl[:, 0:1], mybir.ActivationFunctionType.Tanh
            )
            # broadcast csc[p, t] -> cscw[p, t*128+f] on the idle gpsimd
            # engine (a 5KB/partition csc DMA would delay the A stream)
            for t in range(10):
                nc.gpsimd.tensor_scalar_add(
                    cscw[:, t * 128:(t + 1) * 128],
                    zl[:].bitcast(f32)[:, 0:128],
                    cscs[:, t:t + 1],
                )

            agin = dr.tile([128, SP], f16)
            # the nocc twin's collective stand-in is two local copies (the
            # readback of x1's first half then only waits the first one);
            # Shared space would enforce a single writer
            agout = dr.tile(
                [NCORES * 128, SP], f16,
                addr_space="Local" if nocc else "Shared",
            )

            def fetch_range(b0, b1):
                # ae first: ktile t-order hits the e3m4 tiles (t<4) first
                dste = aer[:, b0 * WE3:b1 * WE3].rearrange(
                    "p (k j) -> p k j", k=b1 - b0
                )
                nc.sync.dma_start(
                    dste, ae[b0:b1].rearrange("k p j -> p k j")
                )
                dstu = aur[:, b0 * WU8:b1 * WU8].rearrange(
                    "p (k j) -> p k j", k=b1 - b0
                )
                nc.sync.dma_start(
                    dstu, au[b0:b1].rearrange("k p j -> p k j")
                )

            def fetch_groups(gi, halves=1):
                """DMA group gi of au + ae on the sync ring."""
                k0 = gi * GSIZE
                bounds = [k0 + (GSIZE * h) // halves for h in range(halves + 1)]
                for b0, b1 in zip(bounds, bounds[1:]):
                    if b0 != b1:
                        fetch_range(b0, b1)

            # rotation weighted by measured cast cost (DVE 445ns, ACT 800ns,
            # GpSimd 1120ns per [128,738] chunk): DVE 3/5, ACT 1/5, Pool 1/5
            cast_engines = (nc.vector, nc.scalar, nc.vector, nc.gpsimd,
                            nc.vector)

            def cast_u8(k, pool, tag, eng=None):
                """u8->f16 cast of AUR ktile k on a rotating engine."""
                st = pool.tile([128, WU8], f16, tag=tag)
                if eng is None:
                    eng = cast_engines[k % 5]
                src = aur[:, k * WU8:(k + 1) * WU8]
                if eng is nc.scalar:
                    nc.scalar.activation(
                        st[:], src, mybir.ActivationFunctionType.Copy
                    )
                else:
                    eng.tensor_copy(st[:], src)
                return st

            # ---- layer 1 (A-stationary; psum is [dst slot, feat]) ----
            # one PSUM tile per 2KiB bank so Tile scopes the eviction's RAW
            # dependency to that bank's stop=True matmul (a single [128,SP]
            # tile would serialize every eviction behind the LAST bank).
            # all PSUM tiles are full 2KiB banks: start=True resets the WHOLE
            # bank, so half-bank tiles sharing a bank would wipe each other
            p1 = [
                ps.tile([128, 512], f32, tag="acc1a", name="p1a"),
                ps.tile([128, 512], f32, tag="acc1b", name="p1b"),
                ps.tile([128, 512], f32, tag="acc1c", name="p1c"),
            ]
            pf = ps.tile([128, 512], f32, tag="pfill", name="pfill")
            # seed each layer-1 bank with one full-width start=True zero
            # matmul: the real matmuls write four 512B ranges per bank with
            # start=False (a per-range start=True would reset the whole bank
            # and erase the sibling ranges' first contributions)
            for pt in p1:
                nc.tensor.matmul(
                    pt[:], zl[:, 0:128], zl[:, 0:512],
                    start=True, stop=False,
                )
            # pre-filler: throwaway matmuls the scheduler hoists to t~1us.
            # They warm the tensor engine's p-state ramp clock and absorb
            # the first stream's ~4.5us DMA latency, so the real layer-1
            # matmuls start at full speed with a stream lead built up —
            # at DMA/PE parity a stall early in layer 1 is never recovered.
            for i in range(prefill):
                nc.tensor.matmul(
                    pf[:], zl[:, 0:128], zl[:, 0:512],
                    start=True, stop=(i == prefill - 1),
                )

            def l1_psum(t):
                b = min(t // 4, 2)
                pt = p1[b][:, (t - b * 4) * 128:(t - b * 4 + 1) * 128]
                if t == 9:
                    # the trimmed last dst tile has only 98 real columns;
                    # partitions 98-127 stay at the seeded zeros
                    pt = pt[0:S - 9 * 128]
                return pt

            def l1_lhsT(k, t, st):
                if t < NE3:
                    return aer[:, k * WE3 + t * 128:k * WE3 + (t + 1) * 128]
                o = (t - NE3) * 128
                return st[:, o:min(o + 128, WU8)]

            kt0 = KT - l1_tail  # start of the t-outer eviction tail
            for gi in range(NG):
                k0, k1 = gi * GSIZE, (gi + 1) * GSIZE
                if k0 == 0:
                    # startup order: first A half-group, tiny x0 chunk, rest
                    # — the first matmul's operands land as early as possible
                    fetch_range(0, 2)
                    nc.sync.dma_start(
                        x0s[:, 0:2 * 128], x0[:, 0:2 * 128]
                    )
                    fetch_range(2, 4)
                    nc.sync.dma_start(
                        x0s[:, 2 * 128:8 * 128], x0[:, 2 * 128:8 * 128]
                    )
                else:
                    if k0 % 8 == 0:
                        # x0 streamed in 8-ktile chunks (fewer DMAs -> less
                        # HWDGE/sem overhead on the shared rings)
                        nc.sync.dma_start(
                            x0s[:, k0 * 128:(k0 + 8) * 128],
                            x0[:, k0 * 128:(k0 + 8) * 128],
                        )
                    # the tail group's casts gate the whole t-outer tail:
                    # fetch it in halves so they start ~1us earlier
                    fetch_groups(gi, halves=2 if k0 >= kt0 else 1)
                if k0 >= kt0:
                    continue
                for k in range(k0, k1):
                    st = cast_u8(k, s1pool, "s1")
                    rhs = x0s[:, k * 128:(k + 1) * 128]
                    for t in range(10):
                        nc.tensor.matmul(
                            l1_psum(t),
                            l1_lhsT(k, t, st),
                            rhs,
                            start=False, stop=False,
                        )

            # evict layer 1: x1 = tanh(cs_dst * psum) [* W1] on ACT (scale is
            # per partition, fused into the activation) into the AllGather
            # bounce, one agin DMA per PSUM bank.
            agin_sb = ev.tile([128, SP], f16, tag="agin")
            ob = ev.tile([128, SP], f32, tag="ob")
            obh = ev.tile([128, SP], f16, tag="obh")

            def evict_l1_bank(b0, b1):
                c0, c1 = b0 * 128, b1 * 128
                b = min(b0 // 4, 2)
                o = (b0 - b * 4) * 128
                nc.vector.tensor_mul(
                    ob[:, c0:c1], p1[b][:, o:o + c1 - c0], cscw[:, c0:c1]
                )
                nc.scalar.activation(
                    agin_sb[:, c0:c1], ob[:, c0:c1],
                    mybir.ActivationFunctionType.Tanh,
                )
                if not w1_ones:
                    for t in range(b0, b1):
                        nc.vector.tensor_mul(
                            agin_sb[:, t * 128:(t + 1) * 128],
                            agin_sb[:, t * 128:(t + 1) * 128], w1s[:],
                        )
                nc.sync.dma_start(agin[:, c0:c1], agin_sb[:, c0:c1])

            # final l1_tail ktiles bank-outer: each dst bank finishes
            # (stop=True) early and its tanh+agin DMA overlap the remaining
            # banks' matmuls; only the last (quarter-size) bank trails.
            # These casts gate the whole tail (its first t-row reads every
            # tail ktile), so pin them to the two fastest engines.
            tail_engs = (nc.vector, nc.scalar, nc.vector, nc.vector)
            sts = {
                k: cast_u8(k, s1pool, "s1", eng=tail_engs[k % 4])
                for k in range(kt0, KT)
            }
            # bank order (b0, b2, b1): the small bank b2 closes mid-tail and
            # its agin DMA clears the HWDGE ring early; the last eviction on
            # the boundary critical path is then a single 512-wide bank
            for b0, b1 in ((0, 4), (8, 10), (4, 8)):
                for t in range(b0, b1):
                    for k in range(kt0, KT):
                        nc.tensor.matmul(
                            l1_psum(t),
                            l1_lhsT(k, t, sts[k]),
                            x0s[:, k * 128:(k + 1) * 128],
                            start=False,
                            stop=(k == KT - 1),
                        )
                evict_l1_bank(b0, b1)

            # layer-2 staging casts have no dependency on the collective:
            # pre-issue a ring's worth so the casters fill them during the
            # layer boundary while the AllGather is in flight.
            s2_pre = {k: cast_u8(k, s2pool, "s2") for k in range(s2bufs)}

            # Keep the PE busy across the layer boundary with throwaway
            # matmuls into a scratch PSUM bank: an idle gap here resets the
            # tensor engine's p-state ramp and the first ~3 us of layer 2
            # would run at half/quarter speed (costs ~9 us).  The filler
            # runs while the AllGather + x1 readback are in flight and is
            # sized to end just before rank 0's x1 lands.
            if filler:
                # anchor on the last eviction bank's tanh output so the
                # scheduler cannot hoist the filler before the boundary
                for i in range(filler):
                    nc.tensor.matmul(
                        pf[:], agin_sb[:, 0:128], zl[:, 0:512],
                        start=True, stop=(i == filler - 1),
                    )

            if nocc:
                nc.sync.dma_start(agout[0:128, 0:512], agin[:, 0:512])
                nc.sync.dma_start(agout[0:128, 512:SP], agin[:, 512:SP])
            else:
                nc.gpsimd.collective_compute(
                    "AllGather",
                    mybir.AluOpType.bypass,
                    replica_groups=[list(range(NCORES))],
                    ins=[agin.opt()],
                    outs=[agout.opt()],
                )
            # agout rank blocks laid side by side in the free dim are exactly
            # layer-2's lhsT tiles in the same padded rank-block order A uses.
            # In the nocc twin only rows 0:128 of agout are written; read all
            # ranks from there so every readback carries the RAW dependency
            # on the collective stand-in (otherwise the sim fires them early
            # and they congest the HWDGE ring right at the layer boundary,
            # which the real program's post-collective readbacks never do).
            # Rank 0 is split so layer 2 restarts on its first ktiles as soon
            # as a quarter-shard lands.
            def ag_src(r):
                return 0 if nocc else r * 128

            nc.sync.dma_start(
                x1s[:, 0:512], agout[ag_src(0):ag_src(0) + 128, 0:512]
            )
            nc.sync.dma_start(
                x1s[:, 512:SP], agout[ag_src(0):ag_src(0) + 128, 512:SP]
            )
            for r in range(1, NCORES):
                nc.sync.dma_start(
                    x1s[:, r * SP:(r + 1) * SP],
                    agout[ag_src(r):ag_src(r) + 128, :],
                )

            # ---- layer 2 (X-stationary; psum is [feat, dst]) ----
            # again one PSUM tile per bank; chunk boundaries are bank-aligned
            # ((0,512) e3m4 direct, (512,512)+(1024,256) casted-f16)
            p2 = [
                ps.tile([128, 512], f32, tag="acc2a", name="p2a"),
                ps.tile([128, 512], f32, tag="acc2b", name="p2b"),
                ps.tile([128, 512], f32, tag="acc2c", name="p2c"),
            ]

            def l2_chunks(k, st):
                yield 0, p2[0][:, 0:512], aer[:, k * WE3:(k + 1) * WE3]
                yield 512, p2[1][:, 0:512], st[:, 0:512]
                yield 1024, p2[2][:, 0:226], st[:, 512:738]

            def l2_cast(k):
                if k in s2_pre:
                    return s2_pre[k]
                return cast_u8(k, s2pool, "s2")

            for k in range(KT - GSIZE):
                st = l2_cast(k)
                lhsT = x1s[:, k * 128:(k + 1) * 128]
                for c0, pt, rhs in l2_chunks(k, st):
                    nc.tensor.matmul(
                        pt[:], lhsT, rhs,
                        start=(k == 0), stop=False,
                    )
            # final group: bank-outer so each psum2 bank completes
            # (stop=True) early and its eviction overlaps the rest
            kf = KT - GSIZE
            sts = {kk: l2_cast(kk) for kk in range(kf, KT)}
            chunks = {kk: list(l2_chunks(kk, sts[kk])) for kk in range(kf, KT)}
            for ci in range(3):
                for kk in range(kf, KT):
                    c0, pt, rhs = chunks[kk][ci]
                    nc.tensor.matmul(
                        pt[:], x1s[:, kk * 128:(kk + 1) * 128], rhs,
                        start=False, stop=(kk == KT - 1),
                    )
                c0, pt, _ = chunks[kf][ci]
                cn = 512 if ci < 2 else 226
                nc.vector.tensor_copy(obh[:, c0:c0 + cn], pt[:])
                nc.sync.dma_start(out[:, c0:c0 + cn], obh[:, c0:c0 + cn])

    nc.compile()
    return nc


def get_program(nocc=False, w1_ones=True, **kw):
    key = (nocc, w1_ones, tuple(sorted(kw.items())))
    if key not in _PROG_CACHE:
        _PROG_CACHE[key] = _build_program(nocc, w1_ones, **kw)
    return _PROG_CACHE[key]


def _node_perm():
    """Padded rank-block src ordering: slot i <-> (rank r = i//1280,
    local q = i%1280); global node r*1250+q for q<1250, else pad."""
    i2 = np.arange(NPAD)
    r2 = i2 // SP
    loc = i2 % SP
    node = r2 * S + loc
    valid = loc < S
    return np.where(valid, node, 0), valid


def build_in_maps(x, src, dst, vals, W):
    """Host-side prep: dense A^T shard (e3m4 + u8 per-column quantized)."""
    import scipy.sparse as sp

    x = np.asarray(x, np.float32)
    src = np.asarray(src, np.int64)
    dst = np.asarray(dst, np.int64)
    vals = np.asarray(vals, np.float32)
    W = np.asarray(W, np.float32)

    # A[dst, src] = sum of vals  ->  we build AT[src, dst]
    AT = sp.coo_matrix((vals, (src, dst)), shape=(N, N)).toarray()

    node2, valid2 = _node_perm()

    xw = x * W[0][None, :]
    x0p = np.zeros((NPAD, D), np.float32)
    x0p[valid2] = xw[node2[valid2]]
    x0h = np.ascontiguousarray(
        x0p.reshape(KT, 128, D).transpose(1, 0, 2).reshape(128, KT * D)
    ).astype(np.float16)

    w1brow = np.ascontiguousarray(
        np.tile(W[1][None, :], (128, 1))
    ).astype(np.float16)

    in_maps = []
    steps = []
    for c in range(NCORES):
        ATc = AT[:, c * S:(c + 1) * S]  # [N, S] float32
        colmax = np.maximum(ATc.max(axis=0), 1e-9)
        # permute + pad src rows once, in f32
        Ap = np.zeros((NPAD, SP), np.float32)
        Ap[valid2, :S] = ATc[node2[valid2]]
        # dequant scale per padded slot
        scale_pad = np.zeros(SP, np.float32)
        cm_pad = np.zeros(SP, np.float32)
        cm_pad[:S] = colmax
        cm_pad[S:] = 1.0
        # e3m4 tiles: slots [0, WE3)
        sc_e3 = E3_TARGET / np.maximum(cm_pad[:WE3], 1e-9)
        Ae = (Ap[:, :WE3] * sc_e3[None, :]).astype(ml_dtypes.float8_e3m4)
        scale_pad[:WE3] = 1.0 / sc_e3
        # u8 tiles: slots [WE3, S) — the pad columns [S, SP) are all-zero
        # and never touched on device
        step = cm_pad[WE3:S] / 255.0
        Au = np.clip(
            np.rint(Ap[:, WE3:S] * (1.0 / step)[None, :]), 0, 255
        ).astype(np.uint8)
        scale_pad[WE3:S] = step
        steps.append(scale_pad)
        # csc[p, t] = dequant scale of dst slot t*128+p
        csc_tile = np.ascontiguousarray(
            scale_pad.reshape(10, 128).T
        ).astype(np.float32)
        in_maps.append(
            {
                "au": np.ascontiguousarray(Au.reshape(KT, 128, WU8)),
                "ae": np.ascontiguousarray(Ae.reshape(KT, 128, WE3)),
                "x0": x0h,
                "csc": csc_tile,
                "w1b": w1brow,
            }
        )
    return in_maps, steps


def assemble_output(results, steps):
    outs = []
    for c in range(NCORES):
        ot = np.asarray(results[c]["out"], np.float32)  # [128, SP] feat-major
        ot = ot * steps[c][None, :]  # per-dst dequant (layer-2)
        outs.append(ot[:, :S].T)
    return np.ascontiguousarray(np.concatenate(outs, axis=0))


def kernel(x, src, dst, vals, W):
    from concourse import bass_utils

    w1_ones = bool(np.all(np.asarray(W)[1] == 1.0))
    nc = get_program(w1_ones=w1_ones)
    in_maps, steps = build_in_maps(x, src, dst, vals, W)
    # The axon terminal can wedge when a different program was loaded
    # earlier in its lifetime; after the crash the terminal restarts and a
    # retry succeeds.  Back off progressively to ride out the restart.
    import time as _time

    last_err = None
    for sleep_s in (10.0, 30.0, 60.0, 0.0):
        try:
            res = bass_utils.run_bass_kernel_spmd(
                nc, in_maps, core_ids=list(range(NCORES))
            )
            return assemble_output(res.results, steps)
        except Exception as e:  # noqa: BLE001
            last_err = e
            _time.sleep(sleep_s)
    raise last_err
